# revision 40
# baseline (speedup 1.0000x reference)
"""Trainium2 8-core Bass kernel for nn_Decoder_Layer_37177236914647.

Decoder layer: self-MHA(+causal mask) -> +res -> LN -> cross-MHA -> +res -> LN
-> FFN(2x dense, no act) -> +res -> LN.  Softmax is over the BATCH axis
(axis=0), faithful to the original model: w[b,h,q,k] = exp(s_b)/sum_b' exp(s_b').
With the reference's fp32 "+ mask*-1e9" the masked positions collapse to
exactly 0.25 for every batch (|scores| << ulp(1e9)=64), reproduced here with a
blend E' = E*(1-m) + m before the batch normalization.

Sharding (v2): attention is head-parallel (16 heads / 8 cores = 2 heads per
core; the batch softmax is local per head).  Activations stay feature-major
([features, tokens]).  Cross-core exchange is ONE fused ReduceScatter:
each core computes Q2 partial products W'[f_c,:]^T x1[f_c,:] for ALL output
features plus its LN1 stats partials (sum, sumsq rows), laid out as 8 blocks
of 130 rows; the ReduceScatter hands core c its 128 Q2 rows (summed over
cores) plus the fully-reduced stats rows.  Q2 is then fixed up analytically:
  Q2 = (W'^T x1 - sw (x) mu) * diag(r)   [+ sbq if be1 != 0]
which equals W'^T LN1(x1).  An AllToAll (bf16) turns the feature-sharded
attn2+res into token-sharded rows for the FFN (512 tokens/core, full
weights); LN2/LN3 local.  Output returned token-sharded, reassembled on host.
"""
import numpy as np
import ml_dtypes

import concourse.bass as bass
import concourse.mybir as mybir
from concourse import bacc
import concourse.tile as tile
from concourse import bass_utils

NC = 8          # cores
B = 4           # batch
S = 1024        # seq len
D = 1024        # d_model
H = 16          # heads
HD = 64         # head dim
F = 128         # features per core (2 heads * 64)
T = B * S       # 4096 flattened tokens
TC = T // NC    # 512 tokens per core (FFN row shard)
NT = T // 512   # 8 token tiles of 512
NF = D // 128   # 8 feature tiles of 128
EPS = 1e-3
P = 128
RSB = F + 2     # reduce-scatter block: 128 Q2 rows + (sum, sumsq)

FP32 = mybir.dt.float32
BF16 = mybir.dt.bfloat16
AX = mybir.AluOpType
AF = mybir.ActivationFunctionType

CLEAN, BOUNDARY, MASKED = 0, 1, 2
_LAST_NC = None
_LAST_IN_MAPS = None


def _emit(nc, tc, io, cls1, bidx, use_cc=True,
          ln_identity=(False, False, False), ffn_bias_zero=False,
          sbq_zero=True):
    from contextlib import ExitStack

    n_bnd = max(bidx.values()) + 1 if bidx else 0
    ctx = ExitStack()
    with ctx:
        # ---- pools (bufs is per-tag N-buffering) ----
        wts = ctx.enter_context(tc.tile_pool(name="wts", bufs=3))
        wq2rp = ctx.enter_context(tc.tile_pool(name="wq2rp", bufs=1))
        srcp = ctx.enter_context(tc.tile_pool(name="srcp", bufs=2))
        scr = ctx.enter_context(tc.tile_pool(name="scr", bufs=2))
        wff = ctx.enter_context(tc.tile_pool(name="wff", bufs=2))
        acts = ctx.enter_context(tc.tile_pool(name="acts", bufs=4))
        epool = ctx.enter_context(tc.tile_pool(name="epool", bufs=3))
        drp = ctx.enter_context(tc.tile_pool(name="drp", bufs=2))
        big = ctx.enter_context(tc.tile_pool(name="big", bufs=2))
        bigh = ctx.enter_context(tc.tile_pool(name="bigh", bufs=2))
        half = ctx.enter_context(tc.tile_pool(name="half", bufs=2))
        stg = ctx.enter_context(tc.tile_pool(name="stg", bufs=1))
        smal = ctx.enter_context(tc.tile_pool(name="smal", bufs=1))
        lns = ctx.enter_context(tc.tile_pool(name="lns", bufs=1))
        abp = ctx.enter_context(tc.tile_pool(name="abp", bufs=1))
        ps = ctx.enter_context(tc.tile_pool(name="ps", bufs=4, space="PSUM"))
        pssc = ctx.enter_context(tc.tile_pool(name="pssc", bufs=2, space="PSUM"))
        dram = ctx.enter_context(tc.tile_pool(name="dram", bufs=1, space="DRAM"))

        # ---- constants ----
        ones_col = smal.tile([P, 1], BF16, tag="onesc")
        nc.vector.memset(ones_col[:], 1.0)
        ones_row = smal.tile([1, P], BF16, tag="onesr")
        nc.vector.memset(ones_row[:], 1.0)
        quarter = smal.tile([P, 512], BF16, tag="quart")
        nc.vector.memset(quarter[:], 0.25)
        eps_col = smal.tile([P, 1], FP32, tag="epsc")
        nc.vector.memset(eps_col[:], EPS)
        eps_row = smal.tile([1, 1], FP32, tag="epsr")
        nc.vector.memset(eps_row[:], EPS)
        zero_col = smal.tile([P, 1], FP32, tag="zeroc")
        nc.vector.memset(zero_col[:], 0.0)
        zero_row = smal.tile([1, 1], FP32, tag="zeror")
        nc.vector.memset(zero_row[:], 0.0)

        if n_bnd:
            mb_sb = smal.tile([P, n_bnd * 512], BF16, tag="mb")
            nc.sync.dma_start(mb_sb[:], io["mbndbar"][:])

        def load_w(name, dt=BF16):
            w = wts.tile([P, NF * 128], dt, tag="w")
            nc.sync.dma_start(w[:, :].rearrange("p (f m) -> p f m", f=NF),
                              io[name].rearrange("(f p) m -> p f m", p=P))
            return w

        def projections(src_ap, w_list, out_dts, has_v):
            """src_ap: [D, T] dram.  w_list: list of weight sbuf tiles; the
            last one is the V weight if has_v.  Returns per-weight outputs:
            QK-style [P, T] and V token-major [P, 32*128]."""
            outs = []
            for wi, (w, dt) in enumerate(zip(w_list, out_dts)):
                outs.append(acts.tile([P, T], dt, tag="act",
                                      name=f"proj_out{wi}"))
            src3 = src_ap.rearrange("(f p) t -> p f t", p=P)
            for j in (0, 2, 4, 6, 1, 3, 5, 7):
                stile = srcp.tile([P, NF, 512], src_ap.dtype, tag="xsrc")
                nc.sync.dma_start(
                    stile[:, :, :],
                    src3[:, :, j * 512:(j + 1) * 512])
                src = [stile[:, f, :] for f in range(NF)]
                nqk = len(w_list) - 1 if has_v else len(w_list)
                for wi in range(nqk):
                    pt = ps.tile([P, 512], FP32, tag="ps512")
                    for f in range(NF):
                        nc.tensor.matmul(
                            pt[:], w_list[wi][:, f * 128:(f + 1) * 128],
                            src[f][:], start=(f == 0), stop=(f == NF - 1))
                    nc.scalar.copy(outs[wi][:, j * 512:(j + 1) * 512], pt[:])
                if has_v:
                    wv = w_list[-1]
                    vout = outs[-1]
                    for i4 in range(4):
                        i = j * 4 + i4
                        pt = ps.tile([P, 512], FP32, tag="ps512")
                        for f in range(NF):
                            nc.tensor.matmul(
                                pt[:, :128],
                                src[f][:, i4 * 128:(i4 + 1) * 128],
                                wv[:, f * 128:(f + 1) * 128],
                                start=(f == 0), stop=(f == NF - 1))
                        nc.vector.tensor_copy(
                            vout[:, i * 128:(i + 1) * 128], pt[:, :128])
            return outs

        def attn_half(QT, KT, V, cls, x_out, res_ap, res_is_sbuf, j,
                      x_bf=None, fillers=None):
            """x_*[:, 1024b+512j : +512] = (sum_k W*V) + res, both heads.
            WV matmuls lag 2 tiles behind scores so the in-order PE never
            waits a full softmax latency per tile (epool bufs=3 keeps the
            lagged W tiles alive)."""
            fillers = fillers if fillers is not None else []
            if True:
                ot = [ps.tile([P, 512], FP32, tag="ps512", name=f"ot{b_}")
                      for b_ in range(4)]
                wv_q = []
                for t in range(8):
                    n_f = (len(fillers) + (8 - t) - 1) // (8 - t)
                    for _ in range(n_f):
                        fillers.pop(0)()
                    tile_cls = cls[t][j]
                    if tile_cls != MASKED:
                        # fully-masked columns (q_local < qc) collapse to
                        # W=0.25 exactly; compute softmax only on [qc:512)
                        qc = 128 * (t % 4) if tile_cls == BOUNDARY else 0
                        Et = epool.tile([P, 2, 4 * 512], BF16, tag="E")
                        e4 = Et[:, :, :].rearrange("p h (c q) -> p h c q", c=4)
                        for qch in range(2):
                            qs = 256 * qch
                            qcl = min(max(qc - qs, 0), 256)
                            if qcl == 256:
                                continue  # chunk fully masked
                            for hh in range(2):
                                # scores for 4 b of q range [qs, qs+256)
                                pt = pssc.tile([P, 4, 256], FP32, tag="sc")
                                for b in range(4):
                                    nc.tensor.matmul(
                                        pt[:, b, :],
                                        KT[64 * hh:64 * (hh + 1),
                                           1024 * b + 128 * t:
                                           1024 * b + 128 * (t + 1)],
                                        QT[64 * hh:64 * (hh + 1),
                                           1024 * b + 512 * j + qs:
                                           1024 * b + 512 * j + qs + 256],
                                        start=True, stop=True)
                                if tile_cls == BOUNDARY:
                                    sl = bidx[(t, j)]
                                    mwid = min(qc + 128, qs + 256) \
                                        - (qs + qcl)
                                    if mwid > 0:
                                        mb = mb_sb[:, sl * 512 + qs + qcl:
                                                   sl * 512 + qs + qcl
                                                   + mwid]
                                        pv = pt[:, :, qcl:qcl + mwid]
                                        nc.vector.tensor_tensor(
                                            pv, pv,
                                            mb[:, None, :].broadcast_to(
                                                [P, 4, mwid]),
                                            op=AX.mult)
                                nc.scalar.activation(
                                    e4[:, hh, :, qs + qcl:qs + 256],
                                    pt[:, :, qcl:],
                                    AF.Exp, bias=zero_col[:])
                                d2 = drp.tile([P, 2, 256], BF16, tag="d2")
                                nc.vector.tensor_tensor(
                                    d2[:, :, qcl:],
                                    e4[:, hh, 0:2, qs + qcl:qs + 256],
                                    e4[:, hh, 2:4, qs + qcl:qs + 256],
                                    op=AX.add)
                                dd = drp.tile([P, 256], BF16, tag="dd")
                                nc.vector.tensor_tensor(
                                    dd[:, qcl:], d2[:, 0, qcl:],
                                    d2[:, 1, qcl:], op=AX.add)
                                rr = drp.tile([P, 256], BF16, tag="rr")
                                with nc.allow_low_precision(
                                        reason="softmax denom ~4, bf16 ok"):
                                    nc.vector.reciprocal(rr[:, qcl:],
                                                         dd[:, qcl:])
                                # W in-place on Et; masked cols [0:qc)
                                # are covered by the quarter-matmul below
                                nc.vector.tensor_tensor(
                                    e4[:, hh, :, qs + qcl:qs + 256],
                                    e4[:, hh, :, qs + qcl:qs + 256],
                                    rr[:, None, qcl:].broadcast_to(
                                        [P, 4, 256 - qcl]),
                                    op=AX.mult)
                    else:
                        qc = 512
                        Et = None

                    def emit_wv(t=t, tile_cls=tile_cls, qc=qc, Et=Et):
                        for b in range(4):
                            for hh in range(2):
                                vsl = V[:, 128 * (8 * b + t) + 64 * hh:
                                           128 * (8 * b + t) + 64 * (hh + 1)]
                                if tile_cls != MASKED and qc < 512:
                                    nc.tensor.matmul(
                                        ot[b][64 * hh:64 * (hh + 1), qc:],
                                        vsl, Et[:, hh, b * 512 + qc:
                                                (b + 1) * 512],
                                        start=(t == 0), stop=(t == 7),
                                        tile_position=(0, 64 * hh))
                                if tile_cls == MASKED or qc > 0:
                                    nc.tensor.matmul(
                                        ot[b][64 * hh:64 * (hh + 1), 0:qc],
                                        vsl, quarter[:, 0:qc],
                                        start=(t == 0), stop=(t == 7),
                                        tile_position=(0, 64 * hh))
                    wv_q.append(emit_wv)
                    if len(wv_q) > 2:
                        wv_q.pop(0)()
                for c_ in wv_q:
                    c_()
                for b in range(4):
                    sl = slice(1024 * b + 512 * j, 1024 * b + 512 * (j + 1))
                    if res_is_sbuf:
                        res = res_ap[:, sl]
                    else:
                        rt = scr.tile([P, 512], FP32, tag="scr")
                        nc.sync.dma_start(rt[:], res_ap[:, sl])
                        res = rt[:]
                    if x_out is not None:
                        nc.vector.tensor_tensor(
                            x_out[:, sl], ot[b][:], res, op=AX.add)
                        if x_bf is not None:
                            nc.scalar.copy(x_bf[:, sl], x_out[:, sl])
                    else:
                        nc.vector.tensor_tensor(
                            x_bf[:, sl], ot[b][:], res, op=AX.add)

        # ================= MHA1 (+ early K2) =================
        wq1 = load_w("wq1s")
        wk1 = load_w("wk1s")
        wv1 = load_w("wv1s")
        QT1, KT1, V1 = projections(io["xdT"], [wq1, wk1, wv1],
                                   [BF16, BF16, BF16], has_v=True)
        wk2 = load_w("wk2s")
        KT2 = acts.tile([P, T], BF16, tag="act", name="KT2")

        wq2r = wq2rp.tile([P, D], BF16, tag="wq2r")
        nc.sync.dma_start(wq2r[:], io["wq2r"][:])
        g1 = smal.tile([P, 1], FP32, tag="g1")
        be1 = smal.tile([P, 1], FP32, tag="be1")
        nc.sync.dma_start(g1[:], io["g1s"][:])
        nc.sync.dma_start(be1[:], io["be1s"][:])
        sw_row = smal.tile([1, P], BF16, tag="swrow")
        nc.sync.dma_start(sw_row[:], io["sw_row"][:])
        sbq = smal.tile([P, 1], FP32, tag="sbq")
        nc.sync.dma_start(sbq[:], io["sbq"][:])

        TH = T // 2
        x1b = half.tile([P, T], BF16, tag="half")
        q2p_d = dram.tile([2, NC * RSB, TH], BF16)
        q2rs_d = dram.tile([2, RSB, TH], BF16)
        m2_d = dram.tile([2, 2, TH], BF16)  # [half, (rr|bneg), half-tokens]

        def stats_partials_closures(h):
            """Closures: stats + Q2 partials for token half h (blocks
            1024*bb + 512*h) -> q2p_d[h] -> ReduceScatter."""
            tcs = [2 * bb + h for bb in range(4)]
            cell = {}

            def stats_one(i, tc_):
                if i == 0:
                    cell["st0"] = lns.tile([1, TH], BF16, tag="strow0", name="st0")
                    cell["st1"] = lns.tile([1, TH], BF16, tag="strow1", name="st1")
                sl = slice(tc_ * 512, (tc_ + 1) * 512)
                so = slice(i * 512, (i + 1) * 512)
                sq = scr.tile([P, 512], BF16, tag="scrb")
                nc.vector.tensor_tensor(sq[:], x1b[:, sl], x1b[:, sl],
                                        op=AX.mult)
                p1 = ps.tile([1, 512], FP32, tag="ps512")
                nc.tensor.matmul(p1[:], ones_col[:], x1b[:, sl],
                                 start=True, stop=True)
                p2 = ps.tile([1, 512], FP32, tag="ps512")
                nc.tensor.matmul(p2[:], ones_col[:], sq[:],
                                 start=True, stop=True)
                nc.scalar.copy(cell["st0"][:, so], p1[:])
                nc.scalar.copy(cell["st1"][:, so], p2[:])

            def part_one(ot_):
                q2st = stg.tile([P, TH], BF16, tag="stg")
                for i, tc_ in enumerate(tcs):
                    pt = ps.tile([P, 512], FP32, tag="ps512")
                    nc.tensor.matmul(
                        pt[:], wq2r[:, ot_ * 128:(ot_ + 1) * 128],
                        x1b[:, tc_ * 512:(tc_ + 1) * 512],
                        start=True, stop=True)
                    if i % 2 == 0:
                        nc.scalar.copy(q2st[:, i * 512:(i + 1) * 512], pt[:])
                    else:
                        nc.vector.tensor_copy(
                            q2st[:, i * 512:(i + 1) * 512], pt[:])
                nc.sync.dma_start(
                    q2p_d[h, ot_ * RSB:ot_ * RSB + P, :], q2st[:])
                nc.sync.dma_start(
                    q2p_d[h, ot_ * RSB + P:ot_ * RSB + P + 1, :],
                    cell["st0"][:])
                nc.sync.dma_start(
                    q2p_d[h, ot_ * RSB + P + 1:(ot_ + 1) * RSB, :],
                    cell["st1"][:])

            def rs():
                if use_cc:
                    nc.gpsimd.collective_compute(
                        "ReduceScatter", AX.add,
                        replica_groups=[list(range(NC))],
                        ins=[q2p_d[h]], outs=[q2rs_d[h]])
                else:
                    nc.sync.dma_start(q2rs_d[h], q2p_d[h, 0:RSB, :])

            cls_ = [lambda i=i, tc_=tc_: stats_one(i, tc_)
                    for i, tc_ in enumerate(tcs)]
            cls_ += [lambda ot_=ot_: part_one(ot_) for ot_ in range(NF)]
            cls_.append(rs)
            return cls_

        def qk_chunk_closures(src_ap, w, out, jlist):
            src3 = src_ap.rearrange("(f p) t -> p f t", p=P)

            def one(j, idx):
                stile = srcp.tile([P, NF, 512], src_ap.dtype, tag="xsrc")
                nc.sync.dma_start(stile[:, :, :],
                                  src3[:, :, j * 512:(j + 1) * 512])
                pt = ps.tile([P, 512], FP32, tag="ps512")
                for f in range(NF):
                    nc.tensor.matmul(
                        pt[:], w[:, f * 128:(f + 1) * 128], stile[:, f, :],
                        start=(f == 0), stop=(f == NF - 1))
                if idx % 2 == 0:
                    nc.scalar.copy(out[:, j * 512:(j + 1) * 512], pt[:])
                else:
                    nc.vector.tensor_copy(out[:, j * 512:(j + 1) * 512],
                                          pt[:])
            return [lambda j=j, idx=idx: one(j, idx)
                    for idx, j in enumerate(jlist)]

        def v_chunk_closures(src_ap, wv, vout, jlist):
            src3 = src_ap.rearrange("(f p) t -> p f t", p=P)

            def one(j):
                stile = srcp.tile([P, NF, 512], src_ap.dtype, tag="xsrc")
                nc.sync.dma_start(stile[:, :, :],
                                  src3[:, :, j * 512:(j + 1) * 512])
                for i4 in range(4):
                    i = j * 4 + i4
                    pt = ps.tile([P, 512], FP32, tag="ps512")
                    for f in range(NF):
                        nc.tensor.matmul(
                            pt[:, :128],
                            stile[:, f, i4 * 128:(i4 + 1) * 128],
                            wv[:, f * 128:(f + 1) * 128],
                            start=(f == 0), stop=(f == NF - 1))
                    if i4 % 2 == 0:
                        nc.scalar.copy(vout[:, i * 128:(i + 1) * 128],
                                       pt[:, :128])
                    else:
                        nc.vector.tensor_copy(
                            vout[:, i * 128:(i + 1) * 128], pt[:, :128])
            return [lambda j=j: one(j) for j in jlist]

        def post_rs(h):
            """mu, r for half h -> m2_d rows (b-major token layout)."""
            s1r = smal.tile([P, 16], BF16, tag="s1r")
            s2r = smal.tile([P, 16], BF16, tag="s2r")
            nc.sync.dma_start(
                s1r[:], q2rs_d[h, P:P + 1, :].rearrange(
                    "o (p i) -> p (o i)", p=P))
            nc.sync.dma_start(
                s2r[:], q2rs_d[h, P + 1:P + 2, :].rearrange(
                    "o (p i) -> p (o i)", p=P))
            mu = smal.tile([P, 16], FP32, tag="mu")
            nc.vector.tensor_scalar_mul(mu[:], s1r[:], 1.0 / D)
            s2f = smal.tile([P, 16], FP32, tag="s2f")
            nc.vector.tensor_scalar_mul(s2f[:], s2r[:], 1.0 / D)
            mu2 = smal.tile([P, 16], FP32, tag="mu2")
            nc.vector.tensor_tensor(mu2[:], mu[:], mu[:], op=AX.mult)
            var = smal.tile([P, 16], FP32, tag="var")
            nc.vector.tensor_tensor(var[:], s2f[:], mu2[:], op=AX.subtract)
            nc.scalar.activation(var[:], var[:], AF.Ln, bias=eps_col[:])
            rr1 = smal.tile([P, 16], FP32, tag="rr1")
            nc.scalar.activation(rr1[:], var[:], AF.Exp, bias=zero_col[:],
                                 scale=-0.5)
            bneg = smal.tile([P, 16], FP32, tag="bneg")
            nc.vector.tensor_tensor(bneg[:], mu[:], rr1[:], op=AX.mult)
            rr1b = smal.tile([P, 16], BF16, tag="rr1b")
            nc.vector.tensor_copy(rr1b[:], rr1[:])
            bnegb = smal.tile([P, 16], BF16, tag="bnegb")
            nc.vector.tensor_copy(bnegb[:], bneg[:])
            nc.sync.dma_start(
                m2_d[h, 0, :].rearrange("(p i) -> p i", p=P), rr1b[:])
            nc.sync.dma_start(
                m2_d[h, 1, :].rearrange("(p i) -> p i", p=P), bnegb[:])

        def fixups(h, a_my, QT2, bbs=None):
            """a_my and QT2 for the four 512-blocks of token half h."""
            for bb in (bbs if bbs is not None else range(4)):
                sl = slice(1024 * bb + 512 * h, 1024 * bb + 512 * (h + 1))
                hs = slice(512 * bb, 512 * (bb + 1))
                a_row = abp.tile([1, 512], BF16, tag="abrow")
                b_row = abp.tile([1, 512], BF16, tag="abrow2")
                nc.sync.dma_start(a_row[:], m2_d[h, 0:1, hs])
                nc.sync.dma_start(b_row[:], m2_d[h, 1:2, hs])
                q2c = scr.tile([P, 512], BF16, tag="scrb2")
                nc.sync.dma_start(
                    q2c[:], q2rs_d[h, 0:P, 512 * bb:512 * (bb + 1)])
                pra = ps.tile([P, 512], FP32, tag="ps512")
                nc.tensor.matmul(pra[:], ones_row[:], a_row[:],
                                 start=True, stop=True)
                prb = ps.tile([P, 512], FP32, tag="ps512")
                nc.tensor.matmul(prb[:], ones_row[:], b_row[:],
                                 start=True, stop=True)
                psw = ps.tile([P, 512], FP32, tag="ps512")
                nc.tensor.matmul(psw[:], sw_row[:], b_row[:],
                                 start=True, stop=True)
                tt = scr.tile([P, 512], FP32, tag="scr")
                nc.vector.tensor_tensor(tt[:], x1b[:, sl], pra[:], op=AX.mult)
                if ln_identity[0]:
                    nc.vector.tensor_tensor(a_my[:, sl], tt[:], prb[:],
                                            op=AX.subtract)
                else:
                    nc.vector.tensor_tensor(tt[:], tt[:], prb[:],
                                            op=AX.subtract)
                    nc.scalar.activation(a_my[:, sl], tt[:], AF.Identity,
                                         bias=be1[:], scale=g1[:])
                tmp = scr.tile([P, 512], BF16, tag="scrb")
                nc.vector.tensor_tensor(tmp[:], q2c[:], pra[:], op=AX.mult)
                if sbq_zero:
                    nc.vector.tensor_tensor(QT2[:, sl], tmp[:], psw[:],
                                            op=AX.subtract)
                else:
                    nc.vector.tensor_tensor(tmp[:], tmp[:], psw[:],
                                            op=AX.subtract)
                    nc.scalar.activation(QT2[:, sl], tmp[:], AF.Identity,
                                         bias=sbq[:], scale=1.0)

        wv2 = load_w("wv2s")
        V2 = acts.tile([P, T], BF16, tag="act", name="V2")
        a_my = big.tile([P, T], FP32, tag="big")
        QT2 = acts.tile([P, T], BF16, tag="act", name="QT2")
        cls_clean = [[CLEAN] * 2 for _ in range(8)]
        x2b = half.tile([P, T], BF16, tag="half")

        # K2 projection before attn1 (KT2 ready for attn2-j0 scores)
        for c_ in qk_chunk_closures(io["xeT"], wk2, KT2,
                                    (0, 2, 4, 6, 1, 3, 5, 7)):
            c_()
        attn_half(QT1, KT1, V1, cls1, None, io["xd_res"], False, 0,
                  x_bf=x1b)
        # half-0 stats/partials -> RS#1 launches while attn1-j1 computes
        for c_ in stats_partials_closures(0):
            c_()
        attn_half(QT1, KT1, V1, cls1, None, io["xd_res"], False, 1,
                  x_bf=x1b)
        for c_ in stats_partials_closures(1):
            c_()
        # V2 projection fills the RS windows
        for c_ in v_chunk_closures(io["xeT"], wv2, V2,
                                   (0, 2, 4, 6, 1, 3, 5, 7)):
            c_()
        post_rs(0)
        fixups(0, a_my, QT2)
        attn_half(QT2, KT2, V2, cls_clean, None, a_my, True, 0, x_bf=x2b)
        post_rs(1)
        fixups(1, a_my, QT2)
        attn_half(QT2, KT2, V2, cls_clean, None, a_my, True, 1, x_bf=x2b)

        # ================= A2A -> token shard (bf16) =================
        a2a_in = dram.tile([D, TC], BF16)
        a2a_out = dram.tile([D, TC], BF16)
        for c_ in range(NC):
            nc.sync.dma_start(a2a_in[128 * c_:128 * (c_ + 1), :],
                              x2b[:, 512 * c_:512 * (c_ + 1)])
        if use_cc:
            nc.gpsimd.collective_compute(
                "AllToAll", AX.bypass, replica_groups=[list(range(NC))],
                ins=[a2a_in[:]], outs=[a2a_out[:]])
        else:
            nc.sync.dma_start(a2a_out[:], a2a_in[:])

        # ================= LN2 / FFN / LN3 (token shard) =================
        def ln_local(get_x, get_xb, g_name, be_name, out_tile,
                     identity=False, out_dma=None):
            """get_x(f): fp32-ish source for normalize; get_xb(f): bf16
            source for stats (may be the same tiles)."""
            sp1 = ps.tile([1, TC], FP32, tag="ps512")
            sp2 = ps.tile([1, TC], FP32, tag="ps512")
            for f in range(NF):
                xb = get_xb(f)
                sq = scr.tile([P, TC], BF16, tag="scrb")
                nc.vector.tensor_tensor(sq[:], xb[:], xb[:], op=AX.mult)
                nc.tensor.matmul(sp1[:], ones_col[:], xb[:],
                                 start=(f == 0), stop=(f == NF - 1))
                nc.tensor.matmul(sp2[:], ones_col[:], sq[:],
                                 start=(f == 0), stop=(f == NF - 1))
            mu_ = lns.tile([1, TC], FP32, tag="lmu")
            nc.vector.tensor_scalar_mul(mu_[:], sp1[:], 1.0 / D)
            var_ = lns.tile([1, TC], FP32, tag="lvar")
            nc.vector.tensor_tensor(var_[:], mu_[:], mu_[:], op=AX.mult)
            nc.vector.scalar_tensor_tensor(var_[:], sp2[:], 1.0 / D, var_[:],
                                           op0=AX.mult, op1=AX.subtract)
            nc.scalar.activation(var_[:], var_[:], AF.Ln, bias=eps_row[:])
            rr_ = lns.tile([1, TC], FP32, tag="lrr")
            nc.scalar.activation(rr_[:], var_[:], AF.Exp, bias=zero_row[:],
                                 scale=-0.5)
            bn_ = lns.tile([1, TC], FP32, tag="lbn")
            nc.vector.tensor_tensor(bn_[:], mu_[:], rr_[:], op=AX.mult)
            rr_b = lns.tile([1, TC], BF16, tag="lrrb")
            nc.vector.tensor_copy(rr_b[:], rr_[:])
            bn_b = lns.tile([1, TC], BF16, tag="lbnb")
            nc.vector.tensor_copy(bn_b[:], bn_[:])
            pra = ps.tile([P, TC], FP32, tag="ps512")
            nc.tensor.matmul(pra[:], ones_row[:], rr_b[:],
                             start=True, stop=True)
            prb = ps.tile([P, TC], FP32, tag="ps512")
            nc.tensor.matmul(prb[:], ones_row[:], bn_b[:],
                             start=True, stop=True)
            if not identity:
                gg = lns.tile([P, NF], FP32, tag="lgg")
                bb = lns.tile([P, NF], FP32, tag="lbb")
                nc.sync.dma_start(
                    gg[:, :, None],
                    io[g_name].rearrange("(f p) o -> p f o", p=P))
                nc.sync.dma_start(
                    bb[:, :, None],
                    io[be_name].rearrange("(f p) o -> p f o", p=P))
            for f in range(NF):
                sl = slice(f * TC, (f + 1) * TC)
                xt = get_x(f)
                tt = scr.tile([P, TC], FP32, tag="scr")
                nc.vector.tensor_tensor(tt[:], xt[:], pra[:], op=AX.mult)
                if identity:
                    nc.vector.tensor_tensor(out_tile[:, sl], tt[:], prb[:],
                                            op=AX.subtract)
                else:
                    nc.vector.tensor_tensor(tt[:], tt[:], prb[:],
                                            op=AX.subtract)
                    nc.scalar.activation(out_tile[:, sl], tt[:], AF.Identity,
                                         bias=bb[:, f:f + 1],
                                         scale=gg[:, f:f + 1])
                if out_dma is not None:
                    out_dma(f)

        x2full = bigh.tile([P, NF * TC], BF16, tag="bigh")
        a2a3 = a2a_out.rearrange("(f p) t -> p f t", p=P)
        x2v = x2full[:, :].rearrange("p (f t) -> p f t", f=NF)
        nc.sync.dma_start(x2v[:, 0:NF // 2, :], a2a3[:, 0:NF // 2, :])
        nc.sync.dma_start(x2v[:, NF // 2:NF, :], a2a3[:, NF // 2:NF, :])
        c_sb = big.tile([P, NF * TC], FP32, tag="big")
        ln_local(lambda f: x2full[:, f * TC:(f + 1) * TC],
                 lambda f: x2full[:, f * TC:(f + 1) * TC],
                 "g2", "be2", c_sb, identity=ln_identity[1])

        h_sb = bigh.tile([P, NF * TC], BF16, tag="bigh")
        bf1 = lns.tile([P, NF], FP32, tag="bf1")
        bf2 = lns.tile([P, NF], FP32, tag="bf2")
        nc.sync.dma_start(bf1[:, :, None],
                          io["bf1"].rearrange("(f p) o -> p f o", p=P))
        nc.sync.dma_start(bf2[:, :, None],
                          io["bf2"].rearrange("(f p) o -> p f o", p=P))
        c_bf = bigh.tile([P, NF * TC], BF16, tag="bigh")
        for f in range(NF):
            nc.vector.tensor_copy(c_bf[:, f * TC:(f + 1) * TC],
                                  c_sb[:, f * TC:(f + 1) * TC])
        for hq in range(NF):
            w1t = wff.tile([P, NF * 128], BF16, tag="wt")
            nc.sync.dma_start(
                w1t[:, :].rearrange("p (f m) -> p f m", f=NF),
                io["w1"][:, hq * 128:(hq + 1) * 128]
                .rearrange("(f p) m -> p f m", p=P))
            pt = ps.tile([P, TC], FP32, tag="ps512")
            for f in range(NF):
                nc.tensor.matmul(pt[:], w1t[:, f * 128:(f + 1) * 128],
                                 c_bf[:, f * TC:(f + 1) * TC],
                                 start=(f == 0), stop=(f == NF - 1))
            if ffn_bias_zero:
                nc.scalar.copy(h_sb[:, hq * TC:(hq + 1) * TC], pt[:])
            else:
                nc.scalar.activation(h_sb[:, hq * TC:(hq + 1) * TC], pt[:],
                                     AF.Identity, bias=bf1[:, hq:hq + 1],
                                     scale=1.0)
        x3 = big.tile([P, NF * TC], FP32, tag="big")
        x3b = half.tile([P, NF * TC], BF16, tag="half")
        for oq in range(NF):
            w2t = wff.tile([P, NF * 128], BF16, tag="wt")
            nc.sync.dma_start(
                w2t[:, :].rearrange("p (f m) -> p f m", f=NF),
                io["w2"][:, oq * 128:(oq + 1) * 128]
                .rearrange("(f p) m -> p f m", p=P))
            pt = ps.tile([P, TC], FP32, tag="ps512")
            for f in range(NF):
                nc.tensor.matmul(pt[:], w2t[:, f * 128:(f + 1) * 128],
                                 h_sb[:, f * TC:(f + 1) * TC],
                                 start=(f == 0), stop=(f == NF - 1))
            sl = slice(oq * TC, (oq + 1) * TC)
            nc.vector.scalar_tensor_tensor(
                x3[:, sl], pt[:], 1.0, c_sb[:, sl],
                op0=AX.mult, op1=AX.add)
            if not ffn_bias_zero:
                nc.scalar.activation(x3[:, sl], x3[:, sl], AF.Identity,
                                     bias=bf2[:, oq:oq + 1], scale=1.0)
            nc.scalar.copy(x3b[:, sl], x3[:, sl])

        y_sb = big.tile([P, NF * TC], FP32, tag="big")
        outv = io["out"].rearrange("(f p) t -> p f t", p=P)
        ln_local(lambda f: x3[:, f * TC:(f + 1) * TC],
                 lambda f: x3b[:, f * TC:(f + 1) * TC],
                 "g3", "be3", y_sb, identity=ln_identity[2],
                 out_dma=lambda f: nc.sync.dma_start(
                     outv[:, f, :], y_sb[:, f * TC:(f + 1) * TC]))


def _build(cls1, bidx, use_cc=True, num_devices=NC,
           ln_identity=(False, False, False), ffn_bias_zero=False,
           sbq_zero=True):
    nc = bacc.Bacc("TRN2", target_bir_lowering=False, debug=False,
                   num_devices=num_devices)
    n_bnd = max(bidx.values()) + 1 if bidx else 0
    io = {}

    def inp(name, shape, dt=FP32):
        io[name] = nc.dram_tensor(name, shape, dt, kind="ExternalInput").ap()

    inp("xdT", [D, T], BF16); inp("xeT", [D, T], BF16); inp("xd_res", [F, T])
    inp("wq1s", [D, F], BF16); inp("wk1s", [D, F], BF16)
    inp("wv1s", [D, F], BF16)
    inp("wq2r", [F, D], BF16)
    inp("sw_row", [1, F], BF16); inp("sbq", [F, 1])
    inp("wk2s", [D, F], BF16); inp("wv2s", [D, F], BF16)
    inp("w1", [D, D], BF16); inp("w2", [D, D], BF16)
    inp("bf1", [D, 1]); inp("bf2", [D, 1])
    inp("g1s", [F, 1]); inp("be1s", [F, 1])
    inp("g2", [D, 1]); inp("be2", [D, 1]); inp("g3", [D, 1]); inp("be3", [D, 1])
    if n_bnd:
        inp("mbndbar", [128, n_bnd * 512], BF16)
    io["out"] = nc.dram_tensor("out", [D, TC], FP32, kind="ExternalOutput").ap()

    with tile.TileContext(nc) as tc:
        _emit(nc, tc, io, cls1, bidx, use_cc=use_cc,
              ln_identity=ln_identity, ffn_bias_zero=ffn_bias_zero,
              sbq_zero=sbq_zero)
    nc.compile()
    return nc


def _classify(mT):
    cls = [[CLEAN] * 2 for _ in range(8)]
    bidx = {}
    for t in range(8):
        for j in range(2):
            sub = mT[128 * t:128 * (t + 1), 512 * j:512 * (j + 1)]
            if sub.max() == 0:
                cls[t][j] = CLEAN
            elif sub.min() == 1:
                cls[t][j] = MASKED
            else:
                cls[t][j] = BOUNDARY
                bidx[(t, j)] = len(bidx)
    return cls, bidx


def kernel(**inputs):
    f32 = np.float32
    bf16 = ml_dtypes.bfloat16
    dec = np.asarray(inputs["dec_input"], f32)
    en = np.asarray(inputs["en_input"], f32)
    lam = np.asarray(inputs["look_ahead_mask"], f32)
    msk2 = np.asarray(inputs["mask"], f32)

    assert np.all(msk2 == 0.0), "cross-attention mask expected to be zero"
    assert np.all((lam == 0.0) | (lam == 1.0)), "mask must be binary"
    assert np.all(lam == lam[0:1]), "mask must be batch-uniform"
    for nm in ("bq1", "bk1", "bv1", "bq2", "bk2", "bv2"):
        assert np.all(np.asarray(inputs[nm]) == 0.0), f"{nm} expected zero"

    mT = np.ascontiguousarray(lam[0, 0].T).astype(f32)  # [k, q]
    cls1, bidx = _classify(mT)
    n_bnd = len(bidx)

    xdT = np.ascontiguousarray(dec.reshape(T, D).T)
    xeT = np.ascontiguousarray(en.reshape(T, D).T)

    mbndbar = np.zeros((128, max(n_bnd, 1) * 512), bf16)
    for (t, j), sl in bidx.items():
        sub = mT[128 * t:128 * (t + 1), 512 * j:512 * (j + 1)]
        mbndbar[:, sl * 512:(sl + 1) * 512] = 1.0 - sub

    Wq1 = np.asarray(inputs["Wq1"], f32); Wk1 = np.asarray(inputs["Wk1"], f32)
    Wv1 = np.asarray(inputs["Wv1"], f32)
    Wq2 = np.asarray(inputs["Wq2"], f32); Wk2 = np.asarray(inputs["Wk2"], f32)
    Wv2 = np.asarray(inputs["Wv2"], f32)
    g1 = np.asarray(inputs["g1"], f32); be1 = np.asarray(inputs["be1"], f32)
    scale = f32(1.0) / np.sqrt(f32(HD))

    # Q2 path host precomputes: W' = diag(g1) Wq2 scale;
    # sw[o] = sum_f W'[f,o]; sbq[o] = sum_f Wq2[f,o] be1[f] scale
    Wq2s = Wq2 * scale
    Wp = Wq2s * g1[:, None]
    sw_full = Wp.sum(axis=0, dtype=f32)            # [D]
    sbq_full = (Wq2s * be1[:, None]).sum(axis=0, dtype=f32)  # [D]

    in_maps = []
    for c in range(NC):
        sl = slice(F * c, F * (c + 1))
        m = {
            "xdT": xdT.astype(bf16), "xeT": xeT.astype(bf16),
            "xd_res": np.ascontiguousarray(xdT[sl]),
            "wq1s": np.ascontiguousarray(Wq1[:, sl] * scale).astype(bf16),
            "wk1s": np.ascontiguousarray(Wk1[:, sl]).astype(bf16),
            "wv1s": np.ascontiguousarray(Wv1[:, sl]).astype(bf16),
            "wq2r": np.ascontiguousarray(Wp[sl, :]).astype(bf16),
            "sw_row": np.ascontiguousarray(sw_full[sl].reshape(1, F))
            .astype(bf16),
            "sbq": np.ascontiguousarray(sbq_full[sl].reshape(F, 1)),
            "wk2s": np.ascontiguousarray(Wk2[:, sl]).astype(bf16),
            "wv2s": np.ascontiguousarray(Wv2[:, sl]).astype(bf16),
            "w1": np.asarray(inputs["W1"], f32).astype(bf16),
            "w2": np.asarray(inputs["W2"], f32).astype(bf16),
            "bf1": np.asarray(inputs["bf1"], f32).reshape(D, 1),
            "bf2": np.asarray(inputs["bf2"], f32).reshape(D, 1),
            "g1s": np.ascontiguousarray(g1[sl].reshape(F, 1)),
            "be1s": np.ascontiguousarray(be1[sl].reshape(F, 1)),
            "g2": np.asarray(inputs["g2"], f32).reshape(D, 1),
            "be2": np.asarray(inputs["be2"], f32).reshape(D, 1),
            "g3": np.asarray(inputs["g3"], f32).reshape(D, 1),
            "be3": np.asarray(inputs["be3"], f32).reshape(D, 1),
        }
        if n_bnd:
            m["mbndbar"] = mbndbar
        in_maps.append(m)

    global _LAST_NC, _LAST_IN_MAPS
    ln_identity = tuple(
        bool(np.all(np.asarray(inputs[g]) == 1.0)
             and np.all(np.asarray(inputs[b]) == 0.0))
        for g, b in (("g1", "be1"), ("g2", "be2"), ("g3", "be3")))
    ffn_bias_zero = bool(np.all(np.asarray(inputs["bf1"]) == 0.0)
                         and np.all(np.asarray(inputs["bf2"]) == 0.0))
    sbq_zero = bool(np.all(sbq_full == 0.0))
    nc = _build(cls1, bidx, ln_identity=ln_identity,
                ffn_bias_zero=ffn_bias_zero, sbq_zero=sbq_zero)
    _LAST_NC, _LAST_IN_MAPS = nc, in_maps
    res = bass_utils.run_bass_kernel_spmd(nc, in_maps, core_ids=list(range(NC)))

    outT = np.empty((D, T), f32)
    for c in range(NC):
        outT[:, TC * c:TC * (c + 1)] = res.results[c]["out"]
    return np.ascontiguousarray(outT.T).reshape(B, S, D).astype(np.float32)


# revision 48
# speedup vs baseline: 1.0631x; 1.0631x over previous
"""Trainium2 8-core Bass kernel for nn_Decoder_Layer_37177236914647.

Decoder layer: self-MHA(+causal mask) -> +res -> LN -> cross-MHA -> +res -> LN
-> FFN(2x dense, no act) -> +res -> LN.  Softmax is over the BATCH axis
(axis=0), faithful to the original model: w[b,h,q,k] = exp(s_b)/sum_b' exp(s_b').
With the reference's fp32 "+ mask*-1e9" the masked positions collapse to
exactly 0.25 for every batch (|scores| << ulp(1e9)=64), reproduced here with a
blend E' = E*(1-m) + m before the batch normalization.

Sharding (v2): attention is head-parallel (16 heads / 8 cores = 2 heads per
core; the batch softmax is local per head).  Activations stay feature-major
([features, tokens]).  Cross-core exchange is ONE fused ReduceScatter:
each core computes Q2 partial products W'[f_c,:]^T x1[f_c,:] for ALL output
features plus its LN1 stats partials (sum, sumsq rows), laid out as 8 blocks
of 130 rows; the ReduceScatter hands core c its 128 Q2 rows (summed over
cores) plus the fully-reduced stats rows.  Q2 is then fixed up analytically:
  Q2 = (W'^T x1 - sw (x) mu) * diag(r)   [+ sbq if be1 != 0]
which equals W'^T LN1(x1).  An AllToAll (bf16) turns the feature-sharded
attn2+res into token-sharded rows for the FFN (512 tokens/core, full
weights); LN2/LN3 local.  Output returned token-sharded, reassembled on host.
"""
import numpy as np
import ml_dtypes

import concourse.bass as bass
import concourse.mybir as mybir
from concourse import bacc
import concourse.tile as tile
from concourse import bass_utils

NC = 8          # cores
B = 4           # batch
S = 1024        # seq len
D = 1024        # d_model
H = 16          # heads
HD = 64         # head dim
F = 128         # features per core (2 heads * 64)
T = B * S       # 4096 flattened tokens
TC = T // NC    # 512 tokens per core (FFN row shard)
NT = T // 512   # 8 token tiles of 512
NF = D // 128   # 8 feature tiles of 128
EPS = 1e-3
P = 128
RSB = F + 2     # reduce-scatter block: 128 Q2 rows + (sum, sumsq)

FP32 = mybir.dt.float32
BF16 = mybir.dt.bfloat16
AX = mybir.AluOpType
AF = mybir.ActivationFunctionType

CLEAN, BOUNDARY, MASKED = 0, 1, 2
_LAST_NC = None
_LAST_IN_MAPS = None


def _emit(nc, tc, io, cls1, bidx, use_cc=True,
          ln_identity=(False, False, False), ffn_bias_zero=False,
          sbq_zero=True):
    from contextlib import ExitStack

    n_bnd = max(bidx.values()) + 1 if bidx else 0
    ctx = ExitStack()
    with ctx:
        # ---- pools (bufs is per-tag N-buffering) ----
        wts = ctx.enter_context(tc.tile_pool(name="wts", bufs=3))
        wq2rp = ctx.enter_context(tc.tile_pool(name="wq2rp", bufs=1))
        srcp = ctx.enter_context(tc.tile_pool(name="srcp", bufs=2))
        scr = ctx.enter_context(tc.tile_pool(name="scr", bufs=2))
        wff = ctx.enter_context(tc.tile_pool(name="wff", bufs=2))
        acts = ctx.enter_context(tc.tile_pool(name="acts", bufs=4))
        epool = ctx.enter_context(tc.tile_pool(name="epool", bufs=3))
        drp = ctx.enter_context(tc.tile_pool(name="drp", bufs=2))
        big = ctx.enter_context(tc.tile_pool(name="big", bufs=2))
        bigh = ctx.enter_context(tc.tile_pool(name="bigh", bufs=2))
        half = ctx.enter_context(tc.tile_pool(name="half", bufs=2))
        stg = ctx.enter_context(tc.tile_pool(name="stg", bufs=1))
        smal = ctx.enter_context(tc.tile_pool(name="smal", bufs=1))
        lns = ctx.enter_context(tc.tile_pool(name="lns", bufs=1))
        abp = ctx.enter_context(tc.tile_pool(name="abp", bufs=1))
        ps = ctx.enter_context(tc.tile_pool(name="ps", bufs=4, space="PSUM"))
        pssc = ctx.enter_context(tc.tile_pool(name="pssc", bufs=2, space="PSUM"))
        dram = ctx.enter_context(tc.tile_pool(name="dram", bufs=1, space="DRAM"))

        # ---- constants ----
        ones_col = smal.tile([P, 1], BF16, tag="onesc")
        nc.vector.memset(ones_col[:], 1.0)
        ones_row = smal.tile([1, P], BF16, tag="onesr")
        nc.vector.memset(ones_row[:], 1.0)
        quarter = smal.tile([P, 512], BF16, tag="quart")
        nc.vector.memset(quarter[:], 0.25)
        eps_col = smal.tile([P, 1], FP32, tag="epsc")
        nc.vector.memset(eps_col[:], EPS)
        eps_row = smal.tile([1, 1], FP32, tag="epsr")
        nc.vector.memset(eps_row[:], EPS)
        zero_col = smal.tile([P, 1], FP32, tag="zeroc")
        nc.vector.memset(zero_col[:], 0.0)
        zero_row = smal.tile([1, 1], FP32, tag="zeror")
        nc.vector.memset(zero_row[:], 0.0)

        if n_bnd:
            mb_sb = smal.tile([P, n_bnd * 512], BF16, tag="mb")
            nc.sync.dma_start(mb_sb[:], io["mbndbar"][:])

        def load_w(name, dt=BF16):
            w = wts.tile([P, NF * 128], dt, tag="w")
            nc.sync.dma_start(w[:, :].rearrange("p (f m) -> p f m", f=NF),
                              io[name].rearrange("(f p) m -> p f m", p=P))
            return w

        def projections(src_ap, w_list, out_dts, has_v):
            """src_ap: [D, T] dram.  w_list: list of weight sbuf tiles; the
            last one is the V weight if has_v.  Returns per-weight outputs:
            QK-style [P, T] and V token-major [P, 32*128]."""
            outs = []
            for wi, (w, dt) in enumerate(zip(w_list, out_dts)):
                outs.append(acts.tile([P, T], dt, tag="act",
                                      name=f"proj_out{wi}"))
            src3 = src_ap.rearrange("(f p) t -> p f t", p=P)
            for j in (0, 2, 4, 6, 1, 3, 5, 7):
                stile = srcp.tile([P, NF, 512], src_ap.dtype, tag="xsrc")
                nc.sync.dma_start(
                    stile[:, :, :],
                    src3[:, :, j * 512:(j + 1) * 512])
                src = [stile[:, f, :] for f in range(NF)]
                nqk = len(w_list) - 1 if has_v else len(w_list)
                for wi in range(nqk):
                    pt = ps.tile([P, 512], FP32, tag="ps512")
                    for f in range(NF):
                        nc.tensor.matmul(
                            pt[:], w_list[wi][:, f * 128:(f + 1) * 128],
                            src[f][:], start=(f == 0), stop=(f == NF - 1))
                    nc.scalar.copy(outs[wi][:, j * 512:(j + 1) * 512], pt[:])
                if has_v:
                    wv = w_list[-1]
                    vout = outs[-1]
                    for i4 in range(4):
                        i = j * 4 + i4
                        pt = ps.tile([P, 512], FP32, tag="ps512")
                        for f in range(NF):
                            nc.tensor.matmul(
                                pt[:, :128],
                                src[f][:, i4 * 128:(i4 + 1) * 128],
                                wv[:, f * 128:(f + 1) * 128],
                                start=(f == 0), stop=(f == NF - 1))
                        nc.vector.tensor_copy(
                            vout[:, i * 128:(i + 1) * 128], pt[:, :128])
            return outs

        def attn_half(QT, KT, V, cls, x_out, res_ap, res_is_sbuf, j,
                      x_bf=None, fillers=None):
            """x_*[:, 1024b+512j : +512] = (sum_k W*V) + res, both heads.
            WV matmuls lag 2 tiles behind scores so the in-order PE never
            waits a full softmax latency per tile (epool bufs=3 keeps the
            lagged W tiles alive)."""
            fillers = fillers if fillers is not None else []
            if True:
                ot = [ps.tile([P, 512], FP32, tag="ps512", name=f"ot{b_}")
                      for b_ in range(4)]
                wv_q = []
                masked_left = sum(1 for t_ in range(8)
                                  if cls[t_][j] == MASKED)
                for t in range(8):
                    tile_cls = cls[t][j]
                    if tile_cls == MASKED and fillers:
                        n_f = (len(fillers) + masked_left - 1) // masked_left
                        for _ in range(min(n_f, len(fillers))):
                            fillers.pop(0)()
                    if tile_cls == MASKED:
                        masked_left -= 1
                    if tile_cls != MASKED:
                        # fully-masked columns (q_local < qc) collapse to
                        # W=0.25 exactly; compute softmax only on [qc:512)
                        qc = 128 * (t % 4) if tile_cls == BOUNDARY else 0
                        Et = epool.tile([P, 2, 4 * 512], BF16, tag="E")
                        e4 = Et[:, :, :].rearrange("p h (c q) -> p h c q", c=4)
                        for qch in range(2):
                            qs = 256 * qch
                            qcl = min(max(qc - qs, 0), 256)
                            if qcl == 256:
                                continue  # chunk fully masked
                            for hh in range(2):
                                # scores for 4 b of q range [qs, qs+256)
                                pt = pssc.tile([P, 4, 256], FP32, tag="sc")
                                for b in range(4):
                                    nc.tensor.matmul(
                                        pt[:, b, :],
                                        KT[64 * hh:64 * (hh + 1),
                                           1024 * b + 128 * t:
                                           1024 * b + 128 * (t + 1)],
                                        QT[64 * hh:64 * (hh + 1),
                                           1024 * b + 512 * j + qs:
                                           1024 * b + 512 * j + qs + 256],
                                        start=True, stop=True)
                                if tile_cls == BOUNDARY:
                                    sl = bidx[(t, j)]
                                    mwid = min(qc + 128, qs + 256) \
                                        - (qs + qcl)
                                    if mwid > 0:
                                        mb = mb_sb[:, sl * 512 + qs + qcl:
                                                   sl * 512 + qs + qcl
                                                   + mwid]
                                        pv = pt[:, :, qcl:qcl + mwid]
                                        nc.vector.tensor_tensor(
                                            pv, pv,
                                            mb[:, None, :].broadcast_to(
                                                [P, 4, mwid]),
                                            op=AX.mult)
                                nc.scalar.activation(
                                    e4[:, hh, :, qs + qcl:qs + 256],
                                    pt[:, :, qcl:],
                                    AF.Exp, bias=zero_col[:])
                                d2 = drp.tile([P, 2, 256], BF16, tag="d2")
                                nc.vector.tensor_tensor(
                                    d2[:, :, qcl:],
                                    e4[:, hh, 0:2, qs + qcl:qs + 256],
                                    e4[:, hh, 2:4, qs + qcl:qs + 256],
                                    op=AX.add)
                                dd = drp.tile([P, 256], BF16, tag="dd")
                                nc.vector.tensor_tensor(
                                    dd[:, qcl:], d2[:, 0, qcl:],
                                    d2[:, 1, qcl:], op=AX.add)
                                rr = drp.tile([P, 256], BF16, tag="rr")
                                with nc.allow_low_precision(
                                        reason="softmax denom ~4, bf16 ok"):
                                    nc.vector.reciprocal(rr[:, qcl:],
                                                         dd[:, qcl:])
                                # W in-place on Et; masked cols [0:qc)
                                # are covered by the quarter-matmul below
                                nc.vector.tensor_tensor(
                                    e4[:, hh, :, qs + qcl:qs + 256],
                                    e4[:, hh, :, qs + qcl:qs + 256],
                                    rr[:, None, qcl:].broadcast_to(
                                        [P, 4, 256 - qcl]),
                                    op=AX.mult)
                    else:
                        qc = 512
                        Et = None

                    def emit_wv(t=t, tile_cls=tile_cls, qc=qc, Et=Et):
                        for b in range(4):
                            for hh in range(2):
                                vsl = V[:, 128 * (8 * b + t) + 64 * hh:
                                           128 * (8 * b + t) + 64 * (hh + 1)]
                                if tile_cls != MASKED and qc < 512:
                                    nc.tensor.matmul(
                                        ot[b][64 * hh:64 * (hh + 1), qc:],
                                        vsl, Et[:, hh, b * 512 + qc:
                                                (b + 1) * 512],
                                        start=(t == 0), stop=(t == 7),
                                        tile_position=(0, 64 * hh))
                                if tile_cls == MASKED or qc > 0:
                                    nc.tensor.matmul(
                                        ot[b][64 * hh:64 * (hh + 1), 0:qc],
                                        vsl, quarter[:, 0:qc],
                                        start=(t == 0), stop=(t == 7),
                                        tile_position=(0, 64 * hh))
                    wv_q.append(emit_wv)
                    if len(wv_q) > 2:
                        wv_q.pop(0)()
                for c_ in fillers:
                    c_()
                del fillers[:]
                for c_ in wv_q:
                    c_()
                for b in range(4):
                    sl = slice(1024 * b + 512 * j, 1024 * b + 512 * (j + 1))
                    if res_is_sbuf:
                        res = res_ap[:, sl]
                    else:
                        rt = scr.tile([P, 512], FP32, tag="scr")
                        nc.sync.dma_start(rt[:], res_ap[:, sl])
                        res = rt[:]
                    if x_out is not None:
                        nc.vector.tensor_tensor(
                            x_out[:, sl], ot[b][:], res, op=AX.add)
                        if x_bf is not None:
                            nc.scalar.copy(x_bf[:, sl], x_out[:, sl])
                    else:
                        nc.vector.tensor_tensor(
                            x_bf[:, sl], ot[b][:], res, op=AX.add)

        # ================= MHA1 (+ early K2) =================
        wq1 = load_w("wq1s")
        wk1 = load_w("wk1s")
        wv1 = load_w("wv1s")
        QT1, KT1, V1 = projections(io["xdT"], [wq1, wk1, wv1],
                                   [BF16, BF16, BF16], has_v=True)
        wk2 = load_w("wk2s")
        KT2 = acts.tile([P, T], BF16, tag="act", name="KT2")

        wq2r = wq2rp.tile([P, D], BF16, tag="wq2r")
        nc.sync.dma_start(wq2r[:], io["wq2r"][:])
        g1 = smal.tile([P, 1], FP32, tag="g1")
        be1 = smal.tile([P, 1], FP32, tag="be1")
        nc.sync.dma_start(g1[:], io["g1s"][:])
        nc.sync.dma_start(be1[:], io["be1s"][:])
        sw_row = smal.tile([1, P], BF16, tag="swrow")
        nc.sync.dma_start(sw_row[:], io["sw_row"][:])
        sbq = smal.tile([P, 1], FP32, tag="sbq")
        nc.sync.dma_start(sbq[:], io["sbq"][:])

        TH = T // 2
        x1b = half.tile([P, T], BF16, tag="half")
        q2p_d = dram.tile([2, NC * RSB, TH], BF16)
        q2rs_d = dram.tile([2, RSB, TH], BF16)
        m2_d = dram.tile([2, 2, TH], BF16)  # [half, (rr|bneg), half-tokens]

        def stats_partials_closures(h):
            """Closures: stats + Q2 partials for token half h (blocks
            1024*bb + 512*h) -> q2p_d[h] -> ReduceScatter."""
            tcs = [2 * bb + h for bb in range(4)]
            cell = {}

            def stats_one(i, tc_):
                if i == 0:
                    cell["st0"] = lns.tile([1, TH], BF16, tag="strow0", name="st0")
                    cell["st1"] = lns.tile([1, TH], BF16, tag="strow1", name="st1")
                sl = slice(tc_ * 512, (tc_ + 1) * 512)
                so = slice(i * 512, (i + 1) * 512)
                sq = scr.tile([P, 512], BF16, tag="scrb")
                nc.vector.tensor_tensor(sq[:], x1b[:, sl], x1b[:, sl],
                                        op=AX.mult)
                p1 = ps.tile([1, 512], FP32, tag="ps512")
                nc.tensor.matmul(p1[:], ones_col[:], x1b[:, sl],
                                 start=True, stop=True)
                p2 = ps.tile([1, 512], FP32, tag="ps512")
                nc.tensor.matmul(p2[:], ones_col[:], sq[:],
                                 start=True, stop=True)
                nc.scalar.copy(cell["st0"][:, so], p1[:])
                nc.scalar.copy(cell["st1"][:, so], p2[:])

            def part_one(ot_):
                q2st = stg.tile([P, TH], BF16, tag="stg")
                for i, tc_ in enumerate(tcs):
                    pt = ps.tile([P, 512], FP32, tag="ps512")
                    nc.tensor.matmul(
                        pt[:], wq2r[:, ot_ * 128:(ot_ + 1) * 128],
                        x1b[:, tc_ * 512:(tc_ + 1) * 512],
                        start=True, stop=True)
                    if i % 2 == 0:
                        nc.scalar.copy(q2st[:, i * 512:(i + 1) * 512], pt[:])
                    else:
                        nc.vector.tensor_copy(
                            q2st[:, i * 512:(i + 1) * 512], pt[:])
                nc.sync.dma_start(
                    q2p_d[h, ot_ * RSB:ot_ * RSB + P, :], q2st[:])
                nc.sync.dma_start(
                    q2p_d[h, ot_ * RSB + P:ot_ * RSB + P + 1, :],
                    cell["st0"][:])
                nc.sync.dma_start(
                    q2p_d[h, ot_ * RSB + P + 1:(ot_ + 1) * RSB, :],
                    cell["st1"][:])

            def rs():
                if use_cc:
                    nc.gpsimd.collective_compute(
                        "ReduceScatter", AX.add,
                        replica_groups=[list(range(NC))],
                        ins=[q2p_d[h]], outs=[q2rs_d[h]])
                else:
                    nc.sync.dma_start(q2rs_d[h], q2p_d[h, 0:RSB, :])

            cls_ = [lambda i=i, tc_=tc_: stats_one(i, tc_)
                    for i, tc_ in enumerate(tcs)]
            cls_ += [lambda ot_=ot_: part_one(ot_) for ot_ in range(NF)]
            cls_.append(rs)
            return cls_

        def qk_chunk_closures(src_ap, w, out, jlist, use_pssc=False):
            src3 = src_ap.rearrange("(f p) t -> p f t", p=P)

            def one(j, idx):
                stile = srcp.tile([P, NF, 512], src_ap.dtype, tag="xsrc")
                nc.sync.dma_start(stile[:, :, :],
                                  src3[:, :, j * 512:(j + 1) * 512])
                if use_pssc:
                    pt = pssc.tile([P, 512], FP32, tag="sc", name="qkpt")
                else:
                    pt = ps.tile([P, 512], FP32, tag="ps512", name="qkpt")
                for f in range(NF):
                    nc.tensor.matmul(
                        pt[:], w[:, f * 128:(f + 1) * 128], stile[:, f, :],
                        start=(f == 0), stop=(f == NF - 1))
                if idx % 2 == 0:
                    nc.scalar.copy(out[:, j * 512:(j + 1) * 512], pt[:])
                else:
                    nc.vector.tensor_copy(out[:, j * 512:(j + 1) * 512],
                                          pt[:])
            return [lambda j=j, idx=idx: one(j, idx)
                    for idx, j in enumerate(jlist)]

        def v_chunk_closures(src_ap, wv, vout, jlist):
            src3 = src_ap.rearrange("(f p) t -> p f t", p=P)

            def one(j):
                stile = srcp.tile([P, NF, 512], src_ap.dtype, tag="xsrc")
                nc.sync.dma_start(stile[:, :, :],
                                  src3[:, :, j * 512:(j + 1) * 512])
                for i4 in range(4):
                    i = j * 4 + i4
                    pt = ps.tile([P, 512], FP32, tag="ps512")
                    for f in range(NF):
                        nc.tensor.matmul(
                            pt[:, :128],
                            stile[:, f, i4 * 128:(i4 + 1) * 128],
                            wv[:, f * 128:(f + 1) * 128],
                            start=(f == 0), stop=(f == NF - 1))
                    if i4 % 2 == 0:
                        nc.scalar.copy(vout[:, i * 128:(i + 1) * 128],
                                       pt[:, :128])
                    else:
                        nc.vector.tensor_copy(
                            vout[:, i * 128:(i + 1) * 128], pt[:, :128])
            return [lambda j=j: one(j) for j in jlist]

        def post_rs(h):
            """mu, r for half h -> m2_d rows (b-major token layout)."""
            s1r = smal.tile([P, 16], BF16, tag="s1r")
            s2r = smal.tile([P, 16], BF16, tag="s2r")
            nc.sync.dma_start(
                s1r[:], q2rs_d[h, P:P + 1, :].rearrange(
                    "o (p i) -> p (o i)", p=P))
            nc.sync.dma_start(
                s2r[:], q2rs_d[h, P + 1:P + 2, :].rearrange(
                    "o (p i) -> p (o i)", p=P))
            mu = smal.tile([P, 16], FP32, tag="mu")
            nc.vector.tensor_scalar_mul(mu[:], s1r[:], 1.0 / D)
            s2f = smal.tile([P, 16], FP32, tag="s2f")
            nc.vector.tensor_scalar_mul(s2f[:], s2r[:], 1.0 / D)
            mu2 = smal.tile([P, 16], FP32, tag="mu2")
            nc.vector.tensor_tensor(mu2[:], mu[:], mu[:], op=AX.mult)
            var = smal.tile([P, 16], FP32, tag="var")
            nc.vector.tensor_tensor(var[:], s2f[:], mu2[:], op=AX.subtract)
            nc.scalar.activation(var[:], var[:], AF.Ln, bias=eps_col[:])
            rr1 = smal.tile([P, 16], FP32, tag="rr1")
            nc.scalar.activation(rr1[:], var[:], AF.Exp, bias=zero_col[:],
                                 scale=-0.5)
            bneg = smal.tile([P, 16], FP32, tag="bneg")
            nc.vector.tensor_tensor(bneg[:], mu[:], rr1[:], op=AX.mult)
            rr1b = smal.tile([P, 16], BF16, tag="rr1b")
            nc.vector.tensor_copy(rr1b[:], rr1[:])
            bnegb = smal.tile([P, 16], BF16, tag="bnegb")
            nc.vector.tensor_copy(bnegb[:], bneg[:])
            nc.sync.dma_start(
                m2_d[h, 0, :].rearrange("(p i) -> p i", p=P), rr1b[:])
            nc.sync.dma_start(
                m2_d[h, 1, :].rearrange("(p i) -> p i", p=P), bnegb[:])

        def fixups(h, a_my, QT2, bbs=None):
            """a_my and QT2 for the four 512-blocks of token half h."""
            for bb in (bbs if bbs is not None else range(4)):
                sl = slice(1024 * bb + 512 * h, 1024 * bb + 512 * (h + 1))
                hs = slice(512 * bb, 512 * (bb + 1))
                a_row = abp.tile([1, 512], BF16, tag="abrow")
                b_row = abp.tile([1, 512], BF16, tag="abrow2")
                nc.sync.dma_start(a_row[:], m2_d[h, 0:1, hs])
                nc.sync.dma_start(b_row[:], m2_d[h, 1:2, hs])
                q2c = scr.tile([P, 512], BF16, tag="scrb2")
                nc.sync.dma_start(
                    q2c[:], q2rs_d[h, 0:P, 512 * bb:512 * (bb + 1)])
                pra = ps.tile([P, 512], FP32, tag="ps512")
                nc.tensor.matmul(pra[:], ones_row[:], a_row[:],
                                 start=True, stop=True)
                prb = ps.tile([P, 512], FP32, tag="ps512")
                nc.tensor.matmul(prb[:], ones_row[:], b_row[:],
                                 start=True, stop=True)
                psw = ps.tile([P, 512], FP32, tag="ps512")
                nc.tensor.matmul(psw[:], sw_row[:], b_row[:],
                                 start=True, stop=True)
                tt = scr.tile([P, 512], FP32, tag="scr")
                nc.vector.tensor_tensor(tt[:], x1b[:, sl], pra[:], op=AX.mult)
                if ln_identity[0]:
                    nc.vector.tensor_tensor(a_my[:, sl], tt[:], prb[:],
                                            op=AX.subtract)
                else:
                    nc.vector.tensor_tensor(tt[:], tt[:], prb[:],
                                            op=AX.subtract)
                    nc.scalar.activation(a_my[:, sl], tt[:], AF.Identity,
                                         bias=be1[:], scale=g1[:])
                tmp = scr.tile([P, 512], BF16, tag="scrb")
                nc.vector.tensor_tensor(tmp[:], q2c[:], pra[:], op=AX.mult)
                if sbq_zero:
                    nc.vector.tensor_tensor(QT2[:, sl], tmp[:], psw[:],
                                            op=AX.subtract)
                else:
                    nc.vector.tensor_tensor(tmp[:], tmp[:], psw[:],
                                            op=AX.subtract)
                    nc.scalar.activation(QT2[:, sl], tmp[:], AF.Identity,
                                         bias=sbq[:], scale=1.0)

        wv2 = load_w("wv2s")
        V2 = acts.tile([P, T], BF16, tag="act", name="V2")
        a_my = big.tile([P, T], FP32, tag="big")
        QT2 = acts.tile([P, T], BF16, tag="act", name="QT2")
        cls_clean = [[CLEAN] * 2 for _ in range(8)]
        x2b = half.tile([P, T], BF16, tag="half")

        # K2 projection interleaved into attn1-j0's masked tiles (those
        # use no score PSUM, so K2 borrows the idle pssc buffers)
        attn_half(QT1, KT1, V1, cls1, None, io["xd_res"], False, 0,
                  x_bf=x1b,
                  fillers=qk_chunk_closures(io["xeT"], wk2, KT2,
                                            (0, 2, 4, 6, 1, 3, 5, 7),
                                            use_pssc=True))
        # half-0 stats/partials -> RS#1 launches while attn1-j1 computes
        for c_ in stats_partials_closures(0):
            c_()
        attn_half(QT1, KT1, V1, cls1, None, io["xd_res"], False, 1,
                  x_bf=x1b)
        for c_ in stats_partials_closures(1):
            c_()
        # V2 projection fills the RS windows
        for c_ in v_chunk_closures(io["xeT"], wv2, V2,
                                   (0, 2, 4, 6, 1, 3, 5, 7)):
            c_()
        post_rs(0)
        fixups(0, a_my, QT2)
        attn_half(QT2, KT2, V2, cls_clean, None, a_my, True, 0, x_bf=x2b)
        post_rs(1)
        fixups(1, a_my, QT2)
        attn_half(QT2, KT2, V2, cls_clean, None, a_my, True, 1, x_bf=x2b)

        # ================= A2A -> token shard (bf16) =================
        a2a_in = dram.tile([D, TC], BF16)
        a2a_out = dram.tile([D, TC], BF16)
        for c_ in range(NC):
            nc.sync.dma_start(a2a_in[128 * c_:128 * (c_ + 1), :],
                              x2b[:, 512 * c_:512 * (c_ + 1)])
        if use_cc:
            nc.gpsimd.collective_compute(
                "AllToAll", AX.bypass, replica_groups=[list(range(NC))],
                ins=[a2a_in[:]], outs=[a2a_out[:]])
        else:
            nc.sync.dma_start(a2a_out[:], a2a_in[:])

        # ================= LN2 / FFN / LN3 (token shard) =================
        def ln_local(get_x, get_xb, g_name, be_name, out_tile,
                     identity=False, out_dma=None):
            """get_x(f): fp32-ish source for normalize; get_xb(f): bf16
            source for stats (may be the same tiles)."""
            sp1 = ps.tile([1, TC], FP32, tag="ps512")
            sp2 = ps.tile([1, TC], FP32, tag="ps512")
            for f in range(NF):
                xb = get_xb(f)
                sq = scr.tile([P, TC], BF16, tag="scrb")
                nc.vector.tensor_tensor(sq[:], xb[:], xb[:], op=AX.mult)
                nc.tensor.matmul(sp1[:], ones_col[:], xb[:],
                                 start=(f == 0), stop=(f == NF - 1))
                nc.tensor.matmul(sp2[:], ones_col[:], sq[:],
                                 start=(f == 0), stop=(f == NF - 1))
            mu_ = lns.tile([1, TC], FP32, tag="lmu")
            nc.vector.tensor_scalar_mul(mu_[:], sp1[:], 1.0 / D)
            var_ = lns.tile([1, TC], FP32, tag="lvar")
            nc.vector.tensor_tensor(var_[:], mu_[:], mu_[:], op=AX.mult)
            nc.vector.scalar_tensor_tensor(var_[:], sp2[:], 1.0 / D, var_[:],
                                           op0=AX.mult, op1=AX.subtract)
            nc.scalar.activation(var_[:], var_[:], AF.Ln, bias=eps_row[:])
            rr_ = lns.tile([1, TC], FP32, tag="lrr")
            nc.scalar.activation(rr_[:], var_[:], AF.Exp, bias=zero_row[:],
                                 scale=-0.5)
            bn_ = lns.tile([1, TC], FP32, tag="lbn")
            nc.vector.tensor_tensor(bn_[:], mu_[:], rr_[:], op=AX.mult)
            rr_b = lns.tile([1, TC], BF16, tag="lrrb")
            nc.vector.tensor_copy(rr_b[:], rr_[:])
            bn_b = lns.tile([1, TC], BF16, tag="lbnb")
            nc.vector.tensor_copy(bn_b[:], bn_[:])
            pra = ps.tile([P, TC], FP32, tag="ps512")
            nc.tensor.matmul(pra[:], ones_row[:], rr_b[:],
                             start=True, stop=True)
            prb = ps.tile([P, TC], FP32, tag="ps512")
            nc.tensor.matmul(prb[:], ones_row[:], bn_b[:],
                             start=True, stop=True)
            if not identity:
                gg = lns.tile([P, NF], FP32, tag="lgg")
                bb = lns.tile([P, NF], FP32, tag="lbb")
                nc.sync.dma_start(
                    gg[:, :, None],
                    io[g_name].rearrange("(f p) o -> p f o", p=P))
                nc.sync.dma_start(
                    bb[:, :, None],
                    io[be_name].rearrange("(f p) o -> p f o", p=P))
            for f in range(NF):
                sl = slice(f * TC, (f + 1) * TC)
                xt = get_x(f)
                tt = scr.tile([P, TC], FP32, tag="scr")
                nc.vector.tensor_tensor(tt[:], xt[:], pra[:], op=AX.mult)
                if identity:
                    nc.vector.tensor_tensor(out_tile[:, sl], tt[:], prb[:],
                                            op=AX.subtract)
                else:
                    nc.vector.tensor_tensor(tt[:], tt[:], prb[:],
                                            op=AX.subtract)
                    nc.scalar.activation(out_tile[:, sl], tt[:], AF.Identity,
                                         bias=bb[:, f:f + 1],
                                         scale=gg[:, f:f + 1])
                if out_dma is not None:
                    out_dma(f)

        x2full = bigh.tile([P, NF * TC], BF16, tag="bigh")
        a2a3 = a2a_out.rearrange("(f p) t -> p f t", p=P)
        x2v = x2full[:, :].rearrange("p (f t) -> p f t", f=NF)
        nc.sync.dma_start(x2v[:, 0:NF // 2, :], a2a3[:, 0:NF // 2, :])
        nc.sync.dma_start(x2v[:, NF // 2:NF, :], a2a3[:, NF // 2:NF, :])
        c_sb = big.tile([P, NF * TC], FP32, tag="big")
        ln_local(lambda f: x2full[:, f * TC:(f + 1) * TC],
                 lambda f: x2full[:, f * TC:(f + 1) * TC],
                 "g2", "be2", c_sb, identity=ln_identity[1])

        h_sb = bigh.tile([P, NF * TC], BF16, tag="bigh")
        bf1 = lns.tile([P, NF], FP32, tag="bf1")
        bf2 = lns.tile([P, NF], FP32, tag="bf2")
        nc.sync.dma_start(bf1[:, :, None],
                          io["bf1"].rearrange("(f p) o -> p f o", p=P))
        nc.sync.dma_start(bf2[:, :, None],
                          io["bf2"].rearrange("(f p) o -> p f o", p=P))
        c_bf = bigh.tile([P, NF * TC], BF16, tag="bigh")
        for f in range(NF):
            nc.vector.tensor_copy(c_bf[:, f * TC:(f + 1) * TC],
                                  c_sb[:, f * TC:(f + 1) * TC])
        for hq in range(NF):
            w1t = wff.tile([P, NF * 128], BF16, tag="wt")
            nc.sync.dma_start(
                w1t[:, :].rearrange("p (f m) -> p f m", f=NF),
                io["w1"][:, hq * 128:(hq + 1) * 128]
                .rearrange("(f p) m -> p f m", p=P))
            pt = ps.tile([P, TC], FP32, tag="ps512")
            for f in range(NF):
                nc.tensor.matmul(pt[:], w1t[:, f * 128:(f + 1) * 128],
                                 c_bf[:, f * TC:(f + 1) * TC],
                                 start=(f == 0), stop=(f == NF - 1))
            if ffn_bias_zero:
                nc.scalar.copy(h_sb[:, hq * TC:(hq + 1) * TC], pt[:])
            else:
                nc.scalar.activation(h_sb[:, hq * TC:(hq + 1) * TC], pt[:],
                                     AF.Identity, bias=bf1[:, hq:hq + 1],
                                     scale=1.0)
        x3 = big.tile([P, NF * TC], FP32, tag="big")
        x3b = half.tile([P, NF * TC], BF16, tag="half")
        for oq in range(NF):
            w2t = wff.tile([P, NF * 128], BF16, tag="wt")
            nc.sync.dma_start(
                w2t[:, :].rearrange("p (f m) -> p f m", f=NF),
                io["w2"][:, oq * 128:(oq + 1) * 128]
                .rearrange("(f p) m -> p f m", p=P))
            pt = ps.tile([P, TC], FP32, tag="ps512")
            for f in range(NF):
                nc.tensor.matmul(pt[:], w2t[:, f * 128:(f + 1) * 128],
                                 h_sb[:, f * TC:(f + 1) * TC],
                                 start=(f == 0), stop=(f == NF - 1))
            sl = slice(oq * TC, (oq + 1) * TC)
            nc.vector.scalar_tensor_tensor(
                x3[:, sl], pt[:], 1.0, c_sb[:, sl],
                op0=AX.mult, op1=AX.add)
            if not ffn_bias_zero:
                nc.scalar.activation(x3[:, sl], x3[:, sl], AF.Identity,
                                     bias=bf2[:, oq:oq + 1], scale=1.0)
            nc.scalar.copy(x3b[:, sl], x3[:, sl])

        y_sb = big.tile([P, NF * TC], FP32, tag="big")
        outv = io["out"].rearrange("(f p) t -> p f t", p=P)
        ln_local(lambda f: x3[:, f * TC:(f + 1) * TC],
                 lambda f: x3b[:, f * TC:(f + 1) * TC],
                 "g3", "be3", y_sb, identity=ln_identity[2],
                 out_dma=lambda f: nc.sync.dma_start(
                     outv[:, f, :], y_sb[:, f * TC:(f + 1) * TC]))


def _build(cls1, bidx, use_cc=True, num_devices=NC,
           ln_identity=(False, False, False), ffn_bias_zero=False,
           sbq_zero=True):
    nc = bacc.Bacc("TRN2", target_bir_lowering=False, debug=False,
                   num_devices=num_devices)
    n_bnd = max(bidx.values()) + 1 if bidx else 0
    io = {}

    def inp(name, shape, dt=FP32):
        io[name] = nc.dram_tensor(name, shape, dt, kind="ExternalInput").ap()

    inp("xdT", [D, T], BF16); inp("xeT", [D, T], BF16); inp("xd_res", [F, T])
    inp("wq1s", [D, F], BF16); inp("wk1s", [D, F], BF16)
    inp("wv1s", [D, F], BF16)
    inp("wq2r", [F, D], BF16)
    inp("sw_row", [1, F], BF16); inp("sbq", [F, 1])
    inp("wk2s", [D, F], BF16); inp("wv2s", [D, F], BF16)
    inp("w1", [D, D], BF16); inp("w2", [D, D], BF16)
    inp("bf1", [D, 1]); inp("bf2", [D, 1])
    inp("g1s", [F, 1]); inp("be1s", [F, 1])
    inp("g2", [D, 1]); inp("be2", [D, 1]); inp("g3", [D, 1]); inp("be3", [D, 1])
    if n_bnd:
        inp("mbndbar", [128, n_bnd * 512], BF16)
    io["out"] = nc.dram_tensor("out", [D, TC], FP32, kind="ExternalOutput").ap()

    with tile.TileContext(nc) as tc:
        _emit(nc, tc, io, cls1, bidx, use_cc=use_cc,
              ln_identity=ln_identity, ffn_bias_zero=ffn_bias_zero,
              sbq_zero=sbq_zero)
    nc.compile()
    return nc


def _classify(mT):
    cls = [[CLEAN] * 2 for _ in range(8)]
    bidx = {}
    for t in range(8):
        for j in range(2):
            sub = mT[128 * t:128 * (t + 1), 512 * j:512 * (j + 1)]
            if sub.max() == 0:
                cls[t][j] = CLEAN
            elif sub.min() == 1:
                cls[t][j] = MASKED
            else:
                cls[t][j] = BOUNDARY
                bidx[(t, j)] = len(bidx)
    return cls, bidx


def kernel(**inputs):
    f32 = np.float32
    bf16 = ml_dtypes.bfloat16
    dec = np.asarray(inputs["dec_input"], f32)
    en = np.asarray(inputs["en_input"], f32)
    lam = np.asarray(inputs["look_ahead_mask"], f32)
    msk2 = np.asarray(inputs["mask"], f32)

    assert np.all(msk2 == 0.0), "cross-attention mask expected to be zero"
    assert np.all((lam == 0.0) | (lam == 1.0)), "mask must be binary"
    assert np.all(lam == lam[0:1]), "mask must be batch-uniform"
    for nm in ("bq1", "bk1", "bv1", "bq2", "bk2", "bv2"):
        assert np.all(np.asarray(inputs[nm]) == 0.0), f"{nm} expected zero"

    mT = np.ascontiguousarray(lam[0, 0].T).astype(f32)  # [k, q]
    cls1, bidx = _classify(mT)
    n_bnd = len(bidx)

    xdT = np.ascontiguousarray(dec.reshape(T, D).T)
    xeT = np.ascontiguousarray(en.reshape(T, D).T)

    mbndbar = np.zeros((128, max(n_bnd, 1) * 512), bf16)
    for (t, j), sl in bidx.items():
        sub = mT[128 * t:128 * (t + 1), 512 * j:512 * (j + 1)]
        mbndbar[:, sl * 512:(sl + 1) * 512] = 1.0 - sub

    Wq1 = np.asarray(inputs["Wq1"], f32); Wk1 = np.asarray(inputs["Wk1"], f32)
    Wv1 = np.asarray(inputs["Wv1"], f32)
    Wq2 = np.asarray(inputs["Wq2"], f32); Wk2 = np.asarray(inputs["Wk2"], f32)
    Wv2 = np.asarray(inputs["Wv2"], f32)
    g1 = np.asarray(inputs["g1"], f32); be1 = np.asarray(inputs["be1"], f32)
    scale = f32(1.0) / np.sqrt(f32(HD))

    # Q2 path host precomputes: W' = diag(g1) Wq2 scale;
    # sw[o] = sum_f W'[f,o]; sbq[o] = sum_f Wq2[f,o] be1[f] scale
    Wq2s = Wq2 * scale
    Wp = Wq2s * g1[:, None]
    sw_full = Wp.sum(axis=0, dtype=f32)            # [D]
    sbq_full = (Wq2s * be1[:, None]).sum(axis=0, dtype=f32)  # [D]

    in_maps = []
    for c in range(NC):
        sl = slice(F * c, F * (c + 1))
        m = {
            "xdT": xdT.astype(bf16), "xeT": xeT.astype(bf16),
            "xd_res": np.ascontiguousarray(xdT[sl]),
            "wq1s": np.ascontiguousarray(Wq1[:, sl] * scale).astype(bf16),
            "wk1s": np.ascontiguousarray(Wk1[:, sl]).astype(bf16),
            "wv1s": np.ascontiguousarray(Wv1[:, sl]).astype(bf16),
            "wq2r": np.ascontiguousarray(Wp[sl, :]).astype(bf16),
            "sw_row": np.ascontiguousarray(sw_full[sl].reshape(1, F))
            .astype(bf16),
            "sbq": np.ascontiguousarray(sbq_full[sl].reshape(F, 1)),
            "wk2s": np.ascontiguousarray(Wk2[:, sl]).astype(bf16),
            "wv2s": np.ascontiguousarray(Wv2[:, sl]).astype(bf16),
            "w1": np.asarray(inputs["W1"], f32).astype(bf16),
            "w2": np.asarray(inputs["W2"], f32).astype(bf16),
            "bf1": np.asarray(inputs["bf1"], f32).reshape(D, 1),
            "bf2": np.asarray(inputs["bf2"], f32).reshape(D, 1),
            "g1s": np.ascontiguousarray(g1[sl].reshape(F, 1)),
            "be1s": np.ascontiguousarray(be1[sl].reshape(F, 1)),
            "g2": np.asarray(inputs["g2"], f32).reshape(D, 1),
            "be2": np.asarray(inputs["be2"], f32).reshape(D, 1),
            "g3": np.asarray(inputs["g3"], f32).reshape(D, 1),
            "be3": np.asarray(inputs["be3"], f32).reshape(D, 1),
        }
        if n_bnd:
            m["mbndbar"] = mbndbar
        in_maps.append(m)

    global _LAST_NC, _LAST_IN_MAPS
    ln_identity = tuple(
        bool(np.all(np.asarray(inputs[g]) == 1.0)
             and np.all(np.asarray(inputs[b]) == 0.0))
        for g, b in (("g1", "be1"), ("g2", "be2"), ("g3", "be3")))
    ffn_bias_zero = bool(np.all(np.asarray(inputs["bf1"]) == 0.0)
                         and np.all(np.asarray(inputs["bf2"]) == 0.0))
    sbq_zero = bool(np.all(sbq_full == 0.0))
    nc = _build(cls1, bidx, ln_identity=ln_identity,
                ffn_bias_zero=ffn_bias_zero, sbq_zero=sbq_zero)
    _LAST_NC, _LAST_IN_MAPS = nc, in_maps
    res = bass_utils.run_bass_kernel_spmd(nc, in_maps, core_ids=list(range(NC)))

    outT = np.empty((D, T), f32)
    for c in range(NC):
        outT[:, TC * c:TC * (c + 1)] = res.results[c]["out"]
    return np.ascontiguousarray(outT.T).reshape(B, S, D).astype(np.float32)


# revision 49
# speedup vs baseline: 2.1198x; 1.9940x over previous
"""Trainium2 8-core Bass kernel for nn_Decoder_Layer_37177236914647.

Decoder layer: self-MHA(+causal mask) -> +res -> LN -> cross-MHA -> +res -> LN
-> FFN(2x dense, no act) -> +res -> LN.  Softmax is over the BATCH axis
(axis=0), faithful to the original model: w[b,h,q,k] = exp(s_b)/sum_b' exp(s_b').
With the reference's fp32 "+ mask*-1e9" the masked positions collapse to
exactly 0.25 for every batch (|scores| << ulp(1e9)=64), reproduced here with a
blend E' = E*(1-m) + m before the batch normalization.

Sharding (v2): attention is head-parallel (16 heads / 8 cores = 2 heads per
core; the batch softmax is local per head).  Activations stay feature-major
([features, tokens]).  Cross-core exchange is ONE fused ReduceScatter:
each core computes Q2 partial products W'[f_c,:]^T x1[f_c,:] for ALL output
features plus its LN1 stats partials (sum, sumsq rows), laid out as 8 blocks
of 130 rows; the ReduceScatter hands core c its 128 Q2 rows (summed over
cores) plus the fully-reduced stats rows.  Q2 is then fixed up analytically:
  Q2 = (W'^T x1 - sw (x) mu) * diag(r)   [+ sbq if be1 != 0]
which equals W'^T LN1(x1).  An AllToAll (bf16) turns the feature-sharded
attn2+res into token-sharded rows for the FFN (512 tokens/core, full
weights); LN2/LN3 local.  Output returned token-sharded, reassembled on host.
"""
import numpy as np
import ml_dtypes

import concourse.bass as bass
import concourse.mybir as mybir
from concourse import bacc
import concourse.tile as tile
from concourse import bass_utils

NC = 8          # cores
B = 4           # batch
S = 1024        # seq len
D = 1024        # d_model
H = 16          # heads
HD = 64         # head dim
F = 128         # features per core (2 heads * 64)
T = B * S       # 4096 flattened tokens
TC = T // NC    # 512 tokens per core (FFN row shard)
NT = T // 512   # 8 token tiles of 512
NF = D // 128   # 8 feature tiles of 128
EPS = 1e-3
P = 128
RSB = F + 2     # reduce-scatter block: 128 Q2 rows + (sum, sumsq)

FP32 = mybir.dt.float32
BF16 = mybir.dt.bfloat16
AX = mybir.AluOpType
AF = mybir.ActivationFunctionType

CLEAN, BOUNDARY, MASKED = 0, 1, 2
_LAST_NC = None
_LAST_IN_MAPS = None


def _emit(nc, tc, io, cls1, bidx, use_cc=True,
          ln_identity=(False, False, False), ffn_bias_zero=False,
          sbq_zero=True):
    from contextlib import ExitStack

    n_bnd = max(bidx.values()) + 1 if bidx else 0
    ctx = ExitStack()
    with ctx:
        # ---- pools (bufs is per-tag N-buffering) ----
        wts = ctx.enter_context(tc.tile_pool(name="wts", bufs=3))
        wq2rp = ctx.enter_context(tc.tile_pool(name="wq2rp", bufs=1))
        srcp = ctx.enter_context(tc.tile_pool(name="srcp", bufs=2))
        scr = ctx.enter_context(tc.tile_pool(name="scr", bufs=2))
        wff = ctx.enter_context(tc.tile_pool(name="wff", bufs=2))
        acts = ctx.enter_context(tc.tile_pool(name="acts", bufs=4))
        epool = ctx.enter_context(tc.tile_pool(name="epool", bufs=3))
        drp = ctx.enter_context(tc.tile_pool(name="drp", bufs=2))
        big = ctx.enter_context(tc.tile_pool(name="big", bufs=2))
        bigh = ctx.enter_context(tc.tile_pool(name="bigh", bufs=2))
        half = ctx.enter_context(tc.tile_pool(name="half", bufs=2))
        stg = ctx.enter_context(tc.tile_pool(name="stg", bufs=1))
        smal = ctx.enter_context(tc.tile_pool(name="smal", bufs=1))
        lns = ctx.enter_context(tc.tile_pool(name="lns", bufs=1))
        abp = ctx.enter_context(tc.tile_pool(name="abp", bufs=1))
        ps = ctx.enter_context(tc.tile_pool(name="ps", bufs=4, space="PSUM"))
        pssc = ctx.enter_context(tc.tile_pool(name="pssc", bufs=2, space="PSUM"))
        dram = ctx.enter_context(tc.tile_pool(name="dram", bufs=1, space="DRAM"))

        # ---- constants ----
        ones_col = smal.tile([P, 1], BF16, tag="onesc")
        nc.vector.memset(ones_col[:], 1.0)
        ones_row = smal.tile([1, P], BF16, tag="onesr")
        nc.vector.memset(ones_row[:], 1.0)
        quarter = smal.tile([P, 512], BF16, tag="quart")
        nc.vector.memset(quarter[:], 0.25)
        eps_col = smal.tile([P, 1], FP32, tag="epsc")
        nc.vector.memset(eps_col[:], EPS)
        eps_row = smal.tile([1, 1], FP32, tag="epsr")
        nc.vector.memset(eps_row[:], EPS)
        zero_col = smal.tile([P, 1], FP32, tag="zeroc")
        nc.vector.memset(zero_col[:], 0.0)
        zero_row = smal.tile([1, 1], FP32, tag="zeror")
        nc.vector.memset(zero_row[:], 0.0)

        if n_bnd:
            mb_sb = smal.tile([P, n_bnd * 512], BF16, tag="mb")
            nc.sync.dma_start(mb_sb[:], io["mbndbar"][:])

        def load_w(name, dt=BF16):
            w = wts.tile([P, NF * 128], dt, tag="w")
            nc.sync.dma_start(w[:, :].rearrange("p (f m) -> p f m", f=NF),
                              io[name].rearrange("(f p) m -> p f m", p=P))
            return w

        def projections(src_ap, w_list, out_dts, has_v):
            """src_ap: [D, T] dram.  w_list: list of weight sbuf tiles; the
            last one is the V weight if has_v.  Returns per-weight outputs:
            QK-style [P, T] and V token-major [P, 32*128]."""
            outs = []
            for wi, (w, dt) in enumerate(zip(w_list, out_dts)):
                outs.append(acts.tile([P, T], dt, tag="act",
                                      name=f"proj_out{wi}"))
            src3 = src_ap.rearrange("(f p) t -> p f t", p=P)
            for j in (0, 2, 4, 6, 1, 3, 5, 7):
                stile = srcp.tile([P, NF, 512], src_ap.dtype, tag="xsrc")
                # two half-DMAs: f=0..3 matmuls start after 0.5MB arrives
                nc.sync.dma_start(
                    stile[:, 0:NF // 2, :],
                    src3[:, 0:NF // 2, j * 512:(j + 1) * 512])
                nc.sync.dma_start(
                    stile[:, NF // 2:NF, :],
                    src3[:, NF // 2:NF, j * 512:(j + 1) * 512])
                src = [stile[:, f, :] for f in range(NF)]
                nqk = len(w_list) - 1 if has_v else len(w_list)
                for wi in range(nqk):
                    pt = ps.tile([P, 512], FP32, tag="ps512")
                    for f in range(NF):
                        nc.tensor.matmul(
                            pt[:], w_list[wi][:, f * 128:(f + 1) * 128],
                            src[f][:], start=(f == 0), stop=(f == NF - 1))
                    nc.scalar.copy(outs[wi][:, j * 512:(j + 1) * 512], pt[:])
                if has_v:
                    wv = w_list[-1]
                    vout = outs[-1]
                    for i4 in range(4):
                        i = j * 4 + i4
                        pt = ps.tile([P, 512], FP32, tag="ps512")
                        for f in range(NF):
                            nc.tensor.matmul(
                                pt[:, :128],
                                src[f][:, i4 * 128:(i4 + 1) * 128],
                                wv[:, f * 128:(f + 1) * 128],
                                start=(f == 0), stop=(f == NF - 1))
                        nc.vector.tensor_copy(
                            vout[:, i * 128:(i + 1) * 128], pt[:, :128])
            return outs

        def attn_half(QT, KT, V, cls, x_out, res_ap, res_is_sbuf, j,
                      x_bf=None, fillers=None):
            """x_*[:, 1024b+512j : +512] = (sum_k W*V) + res, both heads.
            WV matmuls lag 2 tiles behind scores so the in-order PE never
            waits a full softmax latency per tile (epool bufs=3 keeps the
            lagged W tiles alive)."""
            fillers = fillers if fillers is not None else []
            if True:
                ot = [ps.tile([P, 512], FP32, tag="ps512", name=f"ot{b_}")
                      for b_ in range(4)]
                wv_q = []
                masked_left = sum(1 for t_ in range(8)
                                  if cls[t_][j] == MASKED)
                for t in range(8):
                    tile_cls = cls[t][j]
                    if tile_cls == MASKED and fillers:
                        n_f = (len(fillers) + masked_left - 1) // masked_left
                        for _ in range(min(n_f, len(fillers))):
                            fillers.pop(0)()
                    if tile_cls == MASKED:
                        masked_left -= 1
                    if tile_cls != MASKED:
                        # fully-masked columns (q_local < qc) collapse to
                        # W=0.25 exactly; compute softmax only on [qc:512)
                        qc = 128 * (t % 4) if tile_cls == BOUNDARY else 0
                        Et = epool.tile([P, 2, 4 * 512], BF16, tag="E")
                        e4 = Et[:, :, :].rearrange("p h (c q) -> p h c q", c=4)
                        for qch in range(2):
                            qs = 256 * qch
                            qcl = min(max(qc - qs, 0), 256)
                            if qcl == 256:
                                continue  # chunk fully masked
                            for hh in range(2):
                                # scores for 4 b of q range [qs, qs+256)
                                pt = pssc.tile([P, 4, 256], FP32, tag="sc")
                                for b in range(4):
                                    nc.tensor.matmul(
                                        pt[:, b, :],
                                        KT[64 * hh:64 * (hh + 1),
                                           1024 * b + 128 * t:
                                           1024 * b + 128 * (t + 1)],
                                        QT[64 * hh:64 * (hh + 1),
                                           1024 * b + 512 * j + qs:
                                           1024 * b + 512 * j + qs + 256],
                                        start=True, stop=True)
                                if tile_cls == BOUNDARY:
                                    sl = bidx[(t, j)]
                                    mwid = min(qc + 128, qs + 256) \
                                        - (qs + qcl)
                                    if mwid > 0:
                                        mb = mb_sb[:, sl * 512 + qs + qcl:
                                                   sl * 512 + qs + qcl
                                                   + mwid]
                                        pv = pt[:, :, qcl:qcl + mwid]
                                        nc.vector.tensor_tensor(
                                            pv, pv,
                                            mb[:, None, :].broadcast_to(
                                                [P, 4, mwid]),
                                            op=AX.mult)
                                nc.scalar.activation(
                                    e4[:, hh, :, qs + qcl:qs + 256],
                                    pt[:, :, qcl:],
                                    AF.Exp, bias=zero_col[:])
                                d2 = drp.tile([P, 2, 256], BF16, tag="d2")
                                nc.vector.tensor_tensor(
                                    d2[:, :, qcl:],
                                    e4[:, hh, 0:2, qs + qcl:qs + 256],
                                    e4[:, hh, 2:4, qs + qcl:qs + 256],
                                    op=AX.add)
                                dd = drp.tile([P, 256], BF16, tag="dd")
                                nc.vector.tensor_tensor(
                                    dd[:, qcl:], d2[:, 0, qcl:],
                                    d2[:, 1, qcl:], op=AX.add)
                                rr = drp.tile([P, 256], BF16, tag="rr")
                                with nc.allow_low_precision(
                                        reason="softmax denom ~4, bf16 ok"):
                                    nc.vector.reciprocal(rr[:, qcl:],
                                                         dd[:, qcl:])
                                # W in-place on Et; masked cols [0:qc)
                                # are covered by the quarter-matmul below
                                nc.vector.tensor_tensor(
                                    e4[:, hh, :, qs + qcl:qs + 256],
                                    e4[:, hh, :, qs + qcl:qs + 256],
                                    rr[:, None, qcl:].broadcast_to(
                                        [P, 4, 256 - qcl]),
                                    op=AX.mult)
                    else:
                        qc = 512
                        Et = None

                    def emit_wv(t=t, tile_cls=tile_cls, qc=qc, Et=Et):
                        for b in range(4):
                            for hh in range(2):
                                vsl = V[:, 128 * (8 * b + t) + 64 * hh:
                                           128 * (8 * b + t) + 64 * (hh + 1)]
                                if tile_cls != MASKED and qc < 512:
                                    nc.tensor.matmul(
                                        ot[b][64 * hh:64 * (hh + 1), qc:],
                                        vsl, Et[:, hh, b * 512 + qc:
                                                (b + 1) * 512],
                                        start=(t == 0), stop=(t == 7),
                                        tile_position=(0, 64 * hh))
                                if tile_cls == MASKED or qc > 0:
                                    nc.tensor.matmul(
                                        ot[b][64 * hh:64 * (hh + 1), 0:qc],
                                        vsl, quarter[:, 0:qc],
                                        start=(t == 0), stop=(t == 7),
                                        tile_position=(0, 64 * hh))
                    wv_q.append(emit_wv)
                    if len(wv_q) > 2:
                        wv_q.pop(0)()
                for c_ in fillers:
                    c_()
                del fillers[:]
                for c_ in wv_q:
                    c_()
                for b in range(4):
                    sl = slice(1024 * b + 512 * j, 1024 * b + 512 * (j + 1))
                    if res_is_sbuf:
                        res = res_ap[:, sl]
                    else:
                        rt = scr.tile([P, 512], FP32, tag="scr")
                        nc.sync.dma_start(rt[:], res_ap[:, sl])
                        res = rt[:]
                    if x_out is not None:
                        nc.vector.tensor_tensor(
                            x_out[:, sl], ot[b][:], res, op=AX.add)
                        if x_bf is not None:
                            nc.scalar.copy(x_bf[:, sl], x_out[:, sl])
                    else:
                        nc.vector.tensor_tensor(
                            x_bf[:, sl], ot[b][:], res, op=AX.add)

        # ================= MHA1 (+ early K2) =================
        wq1 = load_w("wq1s")
        wk1 = load_w("wk1s")
        wv1 = load_w("wv1s")
        QT1, KT1, V1 = projections(io["xdT"], [wq1, wk1, wv1],
                                   [BF16, BF16, BF16], has_v=True)
        wk2 = load_w("wk2s")
        KT2 = acts.tile([P, T], BF16, tag="act", name="KT2")

        wq2r = wq2rp.tile([P, D], BF16, tag="wq2r")
        nc.sync.dma_start(wq2r[:], io["wq2r"][:])
        g1 = smal.tile([P, 1], FP32, tag="g1")
        be1 = smal.tile([P, 1], FP32, tag="be1")
        nc.sync.dma_start(g1[:], io["g1s"][:])
        nc.sync.dma_start(be1[:], io["be1s"][:])
        sw_row = smal.tile([1, P], BF16, tag="swrow")
        nc.sync.dma_start(sw_row[:], io["sw_row"][:])
        sbq = smal.tile([P, 1], FP32, tag="sbq")
        nc.sync.dma_start(sbq[:], io["sbq"][:])

        TH = T // 2
        x1b = half.tile([P, T], BF16, tag="half")
        q2p_d = dram.tile([2, NC * RSB, TH], BF16)
        q2rs_d = dram.tile([2, RSB, TH], BF16)
        m2_d = dram.tile([2, 2, TH], BF16)  # [half, (rr|bneg), half-tokens]

        def stats_partials_closures(h):
            """Closures: stats + Q2 partials for token half h (blocks
            1024*bb + 512*h) -> q2p_d[h] -> ReduceScatter."""
            tcs = [2 * bb + h for bb in range(4)]
            cell = {}

            def stats_one(i, tc_):
                if i == 0:
                    cell["st0"] = lns.tile([1, TH], BF16, tag="strow0", name="st0")
                    cell["st1"] = lns.tile([1, TH], BF16, tag="strow1", name="st1")
                sl = slice(tc_ * 512, (tc_ + 1) * 512)
                so = slice(i * 512, (i + 1) * 512)
                sq = scr.tile([P, 512], BF16, tag="scrb")
                nc.vector.tensor_tensor(sq[:], x1b[:, sl], x1b[:, sl],
                                        op=AX.mult)
                p1 = ps.tile([1, 512], FP32, tag="ps512")
                nc.tensor.matmul(p1[:], ones_col[:], x1b[:, sl],
                                 start=True, stop=True)
                p2 = ps.tile([1, 512], FP32, tag="ps512")
                nc.tensor.matmul(p2[:], ones_col[:], sq[:],
                                 start=True, stop=True)
                nc.scalar.copy(cell["st0"][:, so], p1[:])
                nc.scalar.copy(cell["st1"][:, so], p2[:])

            def part_one(ot_):
                q2st = stg.tile([P, TH], BF16, tag="stg")
                for i, tc_ in enumerate(tcs):
                    pt = ps.tile([P, 512], FP32, tag="ps512")
                    nc.tensor.matmul(
                        pt[:], wq2r[:, ot_ * 128:(ot_ + 1) * 128],
                        x1b[:, tc_ * 512:(tc_ + 1) * 512],
                        start=True, stop=True)
                    if i % 2 == 0:
                        nc.scalar.copy(q2st[:, i * 512:(i + 1) * 512], pt[:])
                    else:
                        nc.vector.tensor_copy(
                            q2st[:, i * 512:(i + 1) * 512], pt[:])
                nc.sync.dma_start(
                    q2p_d[h, ot_ * RSB:ot_ * RSB + P, :], q2st[:])
                nc.sync.dma_start(
                    q2p_d[h, ot_ * RSB + P:ot_ * RSB + P + 1, :],
                    cell["st0"][:])
                nc.sync.dma_start(
                    q2p_d[h, ot_ * RSB + P + 1:(ot_ + 1) * RSB, :],
                    cell["st1"][:])

            def rs():
                if use_cc:
                    nc.gpsimd.collective_compute(
                        "ReduceScatter", AX.add,
                        replica_groups=[list(range(NC))],
                        ins=[q2p_d[h]], outs=[q2rs_d[h]])
                else:
                    nc.sync.dma_start(q2rs_d[h], q2p_d[h, 0:RSB, :])

            cls_ = [lambda i=i, tc_=tc_: stats_one(i, tc_)
                    for i, tc_ in enumerate(tcs)]
            cls_ += [lambda ot_=ot_: part_one(ot_) for ot_ in range(NF)]
            cls_.append(rs)
            return cls_

        def qk_chunk_closures(src_ap, w, out, jlist, use_pssc=False):
            src3 = src_ap.rearrange("(f p) t -> p f t", p=P)

            def one(j, idx):
                stile = srcp.tile([P, NF, 512], src_ap.dtype, tag="xsrc")
                nc.sync.dma_start(stile[:, :, :],
                                  src3[:, :, j * 512:(j + 1) * 512])
                if use_pssc:
                    pt = pssc.tile([P, 512], FP32, tag="sc", name="qkpt")
                else:
                    pt = ps.tile([P, 512], FP32, tag="ps512", name="qkpt")
                for f in range(NF):
                    nc.tensor.matmul(
                        pt[:], w[:, f * 128:(f + 1) * 128], stile[:, f, :],
                        start=(f == 0), stop=(f == NF - 1))
                if idx % 2 == 0:
                    nc.scalar.copy(out[:, j * 512:(j + 1) * 512], pt[:])
                else:
                    nc.vector.tensor_copy(out[:, j * 512:(j + 1) * 512],
                                          pt[:])
            return [lambda j=j, idx=idx: one(j, idx)
                    for idx, j in enumerate(jlist)]

        def v_chunk_closures(src_ap, wv, vout, jlist):
            src3 = src_ap.rearrange("(f p) t -> p f t", p=P)

            def one(j):
                stile = srcp.tile([P, NF, 512], src_ap.dtype, tag="xsrc")
                nc.sync.dma_start(stile[:, :, :],
                                  src3[:, :, j * 512:(j + 1) * 512])
                for i4 in range(4):
                    i = j * 4 + i4
                    pt = ps.tile([P, 512], FP32, tag="ps512")
                    for f in range(NF):
                        nc.tensor.matmul(
                            pt[:, :128],
                            stile[:, f, i4 * 128:(i4 + 1) * 128],
                            wv[:, f * 128:(f + 1) * 128],
                            start=(f == 0), stop=(f == NF - 1))
                    if i4 % 2 == 0:
                        nc.scalar.copy(vout[:, i * 128:(i + 1) * 128],
                                       pt[:, :128])
                    else:
                        nc.vector.tensor_copy(
                            vout[:, i * 128:(i + 1) * 128], pt[:, :128])
            return [lambda j=j: one(j) for j in jlist]

        def post_rs(h):
            """mu, r for half h -> m2_d rows (b-major token layout)."""
            s1r = smal.tile([P, 16], BF16, tag="s1r")
            s2r = smal.tile([P, 16], BF16, tag="s2r")
            nc.sync.dma_start(
                s1r[:], q2rs_d[h, P:P + 1, :].rearrange(
                    "o (p i) -> p (o i)", p=P))
            nc.sync.dma_start(
                s2r[:], q2rs_d[h, P + 1:P + 2, :].rearrange(
                    "o (p i) -> p (o i)", p=P))
            mu = smal.tile([P, 16], FP32, tag="mu")
            nc.vector.tensor_scalar_mul(mu[:], s1r[:], 1.0 / D)
            s2f = smal.tile([P, 16], FP32, tag="s2f")
            nc.vector.tensor_scalar_mul(s2f[:], s2r[:], 1.0 / D)
            mu2 = smal.tile([P, 16], FP32, tag="mu2")
            nc.vector.tensor_tensor(mu2[:], mu[:], mu[:], op=AX.mult)
            var = smal.tile([P, 16], FP32, tag="var")
            nc.vector.tensor_tensor(var[:], s2f[:], mu2[:], op=AX.subtract)
            nc.scalar.activation(var[:], var[:], AF.Ln, bias=eps_col[:])
            rr1 = smal.tile([P, 16], FP32, tag="rr1")
            nc.scalar.activation(rr1[:], var[:], AF.Exp, bias=zero_col[:],
                                 scale=-0.5)
            bneg = smal.tile([P, 16], FP32, tag="bneg")
            nc.vector.tensor_tensor(bneg[:], mu[:], rr1[:], op=AX.mult)
            rr1b = smal.tile([P, 16], BF16, tag="rr1b")
            nc.vector.tensor_copy(rr1b[:], rr1[:])
            bnegb = smal.tile([P, 16], BF16, tag="bnegb")
            nc.vector.tensor_copy(bnegb[:], bneg[:])
            nc.sync.dma_start(
                m2_d[h, 0, :].rearrange("(p i) -> p i", p=P), rr1b[:])
            nc.sync.dma_start(
                m2_d[h, 1, :].rearrange("(p i) -> p i", p=P), bnegb[:])

        def fixups(h, a_my, QT2, bbs=None):
            """a_my and QT2 for the four 512-blocks of token half h."""
            for bb in (bbs if bbs is not None else range(4)):
                sl = slice(1024 * bb + 512 * h, 1024 * bb + 512 * (h + 1))
                hs = slice(512 * bb, 512 * (bb + 1))
                a_row = abp.tile([1, 512], BF16, tag="abrow")
                b_row = abp.tile([1, 512], BF16, tag="abrow2")
                nc.sync.dma_start(a_row[:], m2_d[h, 0:1, hs])
                nc.sync.dma_start(b_row[:], m2_d[h, 1:2, hs])
                q2c = scr.tile([P, 512], BF16, tag="scrb2")
                nc.sync.dma_start(
                    q2c[:], q2rs_d[h, 0:P, 512 * bb:512 * (bb + 1)])
                pra = ps.tile([P, 512], FP32, tag="ps512")
                nc.tensor.matmul(pra[:], ones_row[:], a_row[:],
                                 start=True, stop=True)
                prb = ps.tile([P, 512], FP32, tag="ps512")
                nc.tensor.matmul(prb[:], ones_row[:], b_row[:],
                                 start=True, stop=True)
                psw = ps.tile([P, 512], FP32, tag="ps512")
                nc.tensor.matmul(psw[:], sw_row[:], b_row[:],
                                 start=True, stop=True)
                tt = scr.tile([P, 512], FP32, tag="scr")
                nc.vector.tensor_tensor(tt[:], x1b[:, sl], pra[:], op=AX.mult)
                if ln_identity[0]:
                    nc.vector.tensor_tensor(a_my[:, sl], tt[:], prb[:],
                                            op=AX.subtract)
                else:
                    nc.vector.tensor_tensor(tt[:], tt[:], prb[:],
                                            op=AX.subtract)
                    nc.scalar.activation(a_my[:, sl], tt[:], AF.Identity,
                                         bias=be1[:], scale=g1[:])
                tmp = scr.tile([P, 512], BF16, tag="scrb")
                nc.vector.tensor_tensor(tmp[:], q2c[:], pra[:], op=AX.mult)
                if sbq_zero:
                    nc.vector.tensor_tensor(QT2[:, sl], tmp[:], psw[:],
                                            op=AX.subtract)
                else:
                    nc.vector.tensor_tensor(tmp[:], tmp[:], psw[:],
                                            op=AX.subtract)
                    nc.scalar.activation(QT2[:, sl], tmp[:], AF.Identity,
                                         bias=sbq[:], scale=1.0)

        wv2 = load_w("wv2s")
        V2 = acts.tile([P, T], BF16, tag="act", name="V2")
        a_my = big.tile([P, T], FP32, tag="big")
        QT2 = acts.tile([P, T], BF16, tag="act", name="QT2")
        cls_clean = [[CLEAN] * 2 for _ in range(8)]
        x2b = half.tile([P, T], BF16, tag="half")

        # K2 projection interleaved into attn1-j0's masked tiles (those
        # use no score PSUM, so K2 borrows the idle pssc buffers)
        attn_half(QT1, KT1, V1, cls1, None, io["xd_res"], False, 0,
                  x_bf=x1b,
                  fillers=qk_chunk_closures(io["xeT"], wk2, KT2,
                                            (0, 2, 4, 6, 1, 3, 5, 7),
                                            use_pssc=True))
        # half-0 stats/partials -> RS#1 launches while attn1-j1 computes
        for c_ in stats_partials_closures(0):
            c_()
        attn_half(QT1, KT1, V1, cls1, None, io["xd_res"], False, 1,
                  x_bf=x1b)
        for c_ in stats_partials_closures(1):
            c_()
        # V2 projection fills the RS windows
        for c_ in v_chunk_closures(io["xeT"], wv2, V2,
                                   (0, 2, 4, 6, 1, 3, 5, 7)):
            c_()
        post_rs(0)
        fixups(0, a_my, QT2)
        attn_half(QT2, KT2, V2, cls_clean, None, a_my, True, 0, x_bf=x2b)
        post_rs(1)
        fixups(1, a_my, QT2)
        attn_half(QT2, KT2, V2, cls_clean, None, a_my, True, 1, x_bf=x2b)

        # ================= A2A -> token shard (bf16) =================
        a2a_in = dram.tile([D, TC], BF16)
        a2a_out = dram.tile([D, TC], BF16)
        for c_ in range(NC):
            nc.sync.dma_start(a2a_in[128 * c_:128 * (c_ + 1), :],
                              x2b[:, 512 * c_:512 * (c_ + 1)])
        if use_cc:
            nc.gpsimd.collective_compute(
                "AllToAll", AX.bypass, replica_groups=[list(range(NC))],
                ins=[a2a_in[:]], outs=[a2a_out[:]])
        else:
            nc.sync.dma_start(a2a_out[:], a2a_in[:])

        # ================= LN2 / FFN / LN3 (token shard) =================
        def ln_local(get_x, get_xb, g_name, be_name, out_tile,
                     identity=False, out_dma=None):
            """get_x(f): fp32-ish source for normalize; get_xb(f): bf16
            source for stats (may be the same tiles)."""
            sp1 = ps.tile([1, TC], FP32, tag="ps512")
            sp2 = ps.tile([1, TC], FP32, tag="ps512")
            for f in range(NF):
                xb = get_xb(f)
                sq = scr.tile([P, TC], BF16, tag="scrb")
                nc.vector.tensor_tensor(sq[:], xb[:], xb[:], op=AX.mult)
                nc.tensor.matmul(sp1[:], ones_col[:], xb[:],
                                 start=(f == 0), stop=(f == NF - 1))
                nc.tensor.matmul(sp2[:], ones_col[:], sq[:],
                                 start=(f == 0), stop=(f == NF - 1))
            mu_ = lns.tile([1, TC], FP32, tag="lmu")
            nc.vector.tensor_scalar_mul(mu_[:], sp1[:], 1.0 / D)
            var_ = lns.tile([1, TC], FP32, tag="lvar")
            nc.vector.tensor_tensor(var_[:], mu_[:], mu_[:], op=AX.mult)
            nc.vector.scalar_tensor_tensor(var_[:], sp2[:], 1.0 / D, var_[:],
                                           op0=AX.mult, op1=AX.subtract)
            nc.scalar.activation(var_[:], var_[:], AF.Ln, bias=eps_row[:])
            rr_ = lns.tile([1, TC], FP32, tag="lrr")
            nc.scalar.activation(rr_[:], var_[:], AF.Exp, bias=zero_row[:],
                                 scale=-0.5)
            bn_ = lns.tile([1, TC], FP32, tag="lbn")
            nc.vector.tensor_tensor(bn_[:], mu_[:], rr_[:], op=AX.mult)
            rr_b = lns.tile([1, TC], BF16, tag="lrrb")
            nc.vector.tensor_copy(rr_b[:], rr_[:])
            bn_b = lns.tile([1, TC], BF16, tag="lbnb")
            nc.vector.tensor_copy(bn_b[:], bn_[:])
            pra = ps.tile([P, TC], FP32, tag="ps512")
            nc.tensor.matmul(pra[:], ones_row[:], rr_b[:],
                             start=True, stop=True)
            prb = ps.tile([P, TC], FP32, tag="ps512")
            nc.tensor.matmul(prb[:], ones_row[:], bn_b[:],
                             start=True, stop=True)
            if not identity:
                gg = lns.tile([P, NF], FP32, tag="lgg")
                bb = lns.tile([P, NF], FP32, tag="lbb")
                nc.sync.dma_start(
                    gg[:, :, None],
                    io[g_name].rearrange("(f p) o -> p f o", p=P))
                nc.sync.dma_start(
                    bb[:, :, None],
                    io[be_name].rearrange("(f p) o -> p f o", p=P))
            for f in range(NF):
                sl = slice(f * TC, (f + 1) * TC)
                xt = get_x(f)
                tt = scr.tile([P, TC], FP32, tag="scr")
                nc.vector.tensor_tensor(tt[:], xt[:], pra[:], op=AX.mult)
                if identity:
                    nc.vector.tensor_tensor(out_tile[:, sl], tt[:], prb[:],
                                            op=AX.subtract)
                else:
                    nc.vector.tensor_tensor(tt[:], tt[:], prb[:],
                                            op=AX.subtract)
                    nc.scalar.activation(out_tile[:, sl], tt[:], AF.Identity,
                                         bias=bb[:, f:f + 1],
                                         scale=gg[:, f:f + 1])
                if out_dma is not None:
                    out_dma(f)

        x2full = bigh.tile([P, NF * TC], BF16, tag="bigh")
        a2a3 = a2a_out.rearrange("(f p) t -> p f t", p=P)
        x2v = x2full[:, :].rearrange("p (f t) -> p f t", f=NF)
        nc.sync.dma_start(x2v[:, 0:NF // 2, :], a2a3[:, 0:NF // 2, :])
        nc.sync.dma_start(x2v[:, NF // 2:NF, :], a2a3[:, NF // 2:NF, :])
        c_sb = big.tile([P, NF * TC], FP32, tag="big")
        ln_local(lambda f: x2full[:, f * TC:(f + 1) * TC],
                 lambda f: x2full[:, f * TC:(f + 1) * TC],
                 "g2", "be2", c_sb, identity=ln_identity[1])

        h_sb = bigh.tile([P, NF * TC], BF16, tag="bigh")
        bf1 = lns.tile([P, NF], FP32, tag="bf1")
        bf2 = lns.tile([P, NF], FP32, tag="bf2")
        nc.sync.dma_start(bf1[:, :, None],
                          io["bf1"].rearrange("(f p) o -> p f o", p=P))
        nc.sync.dma_start(bf2[:, :, None],
                          io["bf2"].rearrange("(f p) o -> p f o", p=P))
        c_bf = bigh.tile([P, NF * TC], BF16, tag="bigh")
        for f in range(NF):
            nc.vector.tensor_copy(c_bf[:, f * TC:(f + 1) * TC],
                                  c_sb[:, f * TC:(f + 1) * TC])
        for hq in range(NF):
            w1t = wff.tile([P, NF * 128], BF16, tag="wt")
            nc.sync.dma_start(
                w1t[:, :].rearrange("p (f m) -> p f m", f=NF),
                io["w1"][:, hq * 128:(hq + 1) * 128]
                .rearrange("(f p) m -> p f m", p=P))
            pt = ps.tile([P, TC], FP32, tag="ps512")
            for f in range(NF):
                nc.tensor.matmul(pt[:], w1t[:, f * 128:(f + 1) * 128],
                                 c_bf[:, f * TC:(f + 1) * TC],
                                 start=(f == 0), stop=(f == NF - 1))
            if ffn_bias_zero:
                nc.scalar.copy(h_sb[:, hq * TC:(hq + 1) * TC], pt[:])
            else:
                nc.scalar.activation(h_sb[:, hq * TC:(hq + 1) * TC], pt[:],
                                     AF.Identity, bias=bf1[:, hq:hq + 1],
                                     scale=1.0)
        x3 = big.tile([P, NF * TC], FP32, tag="big")
        x3b = half.tile([P, NF * TC], BF16, tag="half")
        for oq in range(NF):
            w2t = wff.tile([P, NF * 128], BF16, tag="wt")
            nc.sync.dma_start(
                w2t[:, :].rearrange("p (f m) -> p f m", f=NF),
                io["w2"][:, oq * 128:(oq + 1) * 128]
                .rearrange("(f p) m -> p f m", p=P))
            pt = ps.tile([P, TC], FP32, tag="ps512")
            for f in range(NF):
                nc.tensor.matmul(pt[:], w2t[:, f * 128:(f + 1) * 128],
                                 h_sb[:, f * TC:(f + 1) * TC],
                                 start=(f == 0), stop=(f == NF - 1))
            sl = slice(oq * TC, (oq + 1) * TC)
            nc.vector.scalar_tensor_tensor(
                x3[:, sl], pt[:], 1.0, c_sb[:, sl],
                op0=AX.mult, op1=AX.add)
            if not ffn_bias_zero:
                nc.scalar.activation(x3[:, sl], x3[:, sl], AF.Identity,
                                     bias=bf2[:, oq:oq + 1], scale=1.0)
            nc.scalar.copy(x3b[:, sl], x3[:, sl])

        y_sb = big.tile([P, NF * TC], FP32, tag="big")
        outv = io["out"].rearrange("(f p) t -> p f t", p=P)
        ln_local(lambda f: x3[:, f * TC:(f + 1) * TC],
                 lambda f: x3b[:, f * TC:(f + 1) * TC],
                 "g3", "be3", y_sb, identity=ln_identity[2],
                 out_dma=lambda f: nc.sync.dma_start(
                     outv[:, f, :], y_sb[:, f * TC:(f + 1) * TC]))


def _build(cls1, bidx, use_cc=True, num_devices=NC,
           ln_identity=(False, False, False), ffn_bias_zero=False,
           sbq_zero=True):
    nc = bacc.Bacc("TRN2", target_bir_lowering=False, debug=False,
                   num_devices=num_devices)
    n_bnd = max(bidx.values()) + 1 if bidx else 0
    io = {}

    def inp(name, shape, dt=FP32):
        io[name] = nc.dram_tensor(name, shape, dt, kind="ExternalInput").ap()

    inp("xdT", [D, T], BF16); inp("xeT", [D, T], BF16); inp("xd_res", [F, T])
    inp("wq1s", [D, F], BF16); inp("wk1s", [D, F], BF16)
    inp("wv1s", [D, F], BF16)
    inp("wq2r", [F, D], BF16)
    inp("sw_row", [1, F], BF16); inp("sbq", [F, 1])
    inp("wk2s", [D, F], BF16); inp("wv2s", [D, F], BF16)
    inp("w1", [D, D], BF16); inp("w2", [D, D], BF16)
    inp("bf1", [D, 1]); inp("bf2", [D, 1])
    inp("g1s", [F, 1]); inp("be1s", [F, 1])
    inp("g2", [D, 1]); inp("be2", [D, 1]); inp("g3", [D, 1]); inp("be3", [D, 1])
    if n_bnd:
        inp("mbndbar", [128, n_bnd * 512], BF16)
    io["out"] = nc.dram_tensor("out", [D, TC], FP32, kind="ExternalOutput").ap()

    with tile.TileContext(nc) as tc:
        _emit(nc, tc, io, cls1, bidx, use_cc=use_cc,
              ln_identity=ln_identity, ffn_bias_zero=ffn_bias_zero,
              sbq_zero=sbq_zero)
    nc.compile()
    return nc


def _classify(mT):
    cls = [[CLEAN] * 2 for _ in range(8)]
    bidx = {}
    for t in range(8):
        for j in range(2):
            sub = mT[128 * t:128 * (t + 1), 512 * j:512 * (j + 1)]
            if sub.max() == 0:
                cls[t][j] = CLEAN
            elif sub.min() == 1:
                cls[t][j] = MASKED
            else:
                cls[t][j] = BOUNDARY
                bidx[(t, j)] = len(bidx)
    return cls, bidx


def kernel(**inputs):
    f32 = np.float32
    bf16 = ml_dtypes.bfloat16
    dec = np.asarray(inputs["dec_input"], f32)
    en = np.asarray(inputs["en_input"], f32)
    lam = np.asarray(inputs["look_ahead_mask"], f32)
    msk2 = np.asarray(inputs["mask"], f32)

    assert np.all(msk2 == 0.0), "cross-attention mask expected to be zero"
    assert np.all((lam == 0.0) | (lam == 1.0)), "mask must be binary"
    assert np.all(lam == lam[0:1]), "mask must be batch-uniform"
    for nm in ("bq1", "bk1", "bv1", "bq2", "bk2", "bv2"):
        assert np.all(np.asarray(inputs[nm]) == 0.0), f"{nm} expected zero"

    mT = np.ascontiguousarray(lam[0, 0].T).astype(f32)  # [k, q]
    cls1, bidx = _classify(mT)
    n_bnd = len(bidx)

    xdT = np.ascontiguousarray(dec.reshape(T, D).T)
    xeT = np.ascontiguousarray(en.reshape(T, D).T)

    mbndbar = np.zeros((128, max(n_bnd, 1) * 512), bf16)
    for (t, j), sl in bidx.items():
        sub = mT[128 * t:128 * (t + 1), 512 * j:512 * (j + 1)]
        mbndbar[:, sl * 512:(sl + 1) * 512] = 1.0 - sub

    Wq1 = np.asarray(inputs["Wq1"], f32); Wk1 = np.asarray(inputs["Wk1"], f32)
    Wv1 = np.asarray(inputs["Wv1"], f32)
    Wq2 = np.asarray(inputs["Wq2"], f32); Wk2 = np.asarray(inputs["Wk2"], f32)
    Wv2 = np.asarray(inputs["Wv2"], f32)
    g1 = np.asarray(inputs["g1"], f32); be1 = np.asarray(inputs["be1"], f32)
    scale = f32(1.0) / np.sqrt(f32(HD))

    # Q2 path host precomputes: W' = diag(g1) Wq2 scale;
    # sw[o] = sum_f W'[f,o]; sbq[o] = sum_f Wq2[f,o] be1[f] scale
    Wq2s = Wq2 * scale
    Wp = Wq2s * g1[:, None]
    sw_full = Wp.sum(axis=0, dtype=f32)            # [D]
    sbq_full = (Wq2s * be1[:, None]).sum(axis=0, dtype=f32)  # [D]

    in_maps = []
    for c in range(NC):
        sl = slice(F * c, F * (c + 1))
        m = {
            "xdT": xdT.astype(bf16), "xeT": xeT.astype(bf16),
            "xd_res": np.ascontiguousarray(xdT[sl]),
            "wq1s": np.ascontiguousarray(Wq1[:, sl] * scale).astype(bf16),
            "wk1s": np.ascontiguousarray(Wk1[:, sl]).astype(bf16),
            "wv1s": np.ascontiguousarray(Wv1[:, sl]).astype(bf16),
            "wq2r": np.ascontiguousarray(Wp[sl, :]).astype(bf16),
            "sw_row": np.ascontiguousarray(sw_full[sl].reshape(1, F))
            .astype(bf16),
            "sbq": np.ascontiguousarray(sbq_full[sl].reshape(F, 1)),
            "wk2s": np.ascontiguousarray(Wk2[:, sl]).astype(bf16),
            "wv2s": np.ascontiguousarray(Wv2[:, sl]).astype(bf16),
            "w1": np.asarray(inputs["W1"], f32).astype(bf16),
            "w2": np.asarray(inputs["W2"], f32).astype(bf16),
            "bf1": np.asarray(inputs["bf1"], f32).reshape(D, 1),
            "bf2": np.asarray(inputs["bf2"], f32).reshape(D, 1),
            "g1s": np.ascontiguousarray(g1[sl].reshape(F, 1)),
            "be1s": np.ascontiguousarray(be1[sl].reshape(F, 1)),
            "g2": np.asarray(inputs["g2"], f32).reshape(D, 1),
            "be2": np.asarray(inputs["be2"], f32).reshape(D, 1),
            "g3": np.asarray(inputs["g3"], f32).reshape(D, 1),
            "be3": np.asarray(inputs["be3"], f32).reshape(D, 1),
        }
        if n_bnd:
            m["mbndbar"] = mbndbar
        in_maps.append(m)

    global _LAST_NC, _LAST_IN_MAPS
    ln_identity = tuple(
        bool(np.all(np.asarray(inputs[g]) == 1.0)
             and np.all(np.asarray(inputs[b]) == 0.0))
        for g, b in (("g1", "be1"), ("g2", "be2"), ("g3", "be3")))
    ffn_bias_zero = bool(np.all(np.asarray(inputs["bf1"]) == 0.0)
                         and np.all(np.asarray(inputs["bf2"]) == 0.0))
    sbq_zero = bool(np.all(sbq_full == 0.0))
    nc = _build(cls1, bidx, ln_identity=ln_identity,
                ffn_bias_zero=ffn_bias_zero, sbq_zero=sbq_zero)
    _LAST_NC, _LAST_IN_MAPS = nc, in_maps
    res = bass_utils.run_bass_kernel_spmd(nc, in_maps, core_ids=list(range(NC)))

    outT = np.empty((D, T), f32)
    for c in range(NC):
        outT[:, TC * c:TC * (c + 1)] = res.results[c]["out"]
    return np.ascontiguousarray(outT.T).reshape(B, S, D).astype(np.float32)
